# revision 1
# baseline (speedup 1.0000x reference)
"""CrossModalFeatureInteraction kernel for Trainium2 (Bass/Tile), 8 NeuronCores.

Computation (per pixel, per batch):
    combined = concat([vis, ir], channel)              # [512]
    x        = relu(W1 @ combined + b1)                # [32]
    residual = W2 @ x + b2                             # [256]
    out      = vis + ir + residual                     # [256]

Sharding: data-parallel over batch. B=16 -> 2 images per core on 8 cores.
Weights are tiny and replicated. Each core streams its 2 images through
SBUF in pixel supertiles; 1x1 convs are matmuls with channels as the
contraction dim and pixels as the moving free dim.

Engine budget tricks (target regime is memory; DMA ~77us/core is the
roofline, so every other engine must stay well under it):
  - Matmuls run in float32r: full-rate (1 col/cycle) PE mode on fp32 bits.
  - b1 rides as the activation bias; an all-zero 33rd W1 column plus
    bias=1.0 makes x's 33rd row == 1.0, so b2 rides as the 33rd row of
    W2 (K=33 second matmul). No separate bias pass.
  - Each supertile runs in two phases: all first-layer matmuls + relus
    into one batched x tile first, then all second-layer matmuls. The
    PE never waits on the ACT relu this way (its consumer runs ~4 tile
    slots behind the producer), so it stays dense and HAM-warm.
  - variant "hybrid": the ir half of the bypass is added by the PE (an
    identity matmul accumulating into the residual PSUM bank), leaving
    DVE one add per output tile. variant "dve": both bypass adds on DVE
    (fp32 tensor_tensor is 1x mode, so this doubles DVE time but frees
    the PE).
"""

import numpy as np

import concourse.bass as bass
import concourse.mybir as mybir
from concourse import bacc, bass_utils
from concourse.tile import TileContext

# Problem shape (hardcoded per contract)
B, C, H, W = 16, 256, 64, 64
HID = 32
HWPIX = H * W          # 4096 pixels per image
N_CORES = 8
B_PER_CORE = B // N_CORES  # 2

import os
NBIG = int(os.environ.get("KNBIG", "2048"))  # pixels per DMA supertile (1 MiB tiles)
INBUFS = int(os.environ.get("KINBUFS", "4"))  # input-tile double-buffer depth
NT = 512               # matmul moving free dim (one PSUM bank of fp32)
KO = 4                 # 512 combined channels / 128 partitions
CCH = 2                # 256 output channels / 128 partitions
HID1 = HID + 1         # hidden + ones row (carries b2 through matmul 2)

F32 = mybir.dt.float32
F32R = mybir.dt.float32r

_cache = {}


def _build(mm_dt: str, variant: str) -> bass.Bass:
    d = F32R if mm_dt == "f32r" else F32
    hybrid = variant == "hybrid"

    nc = bacc.Bacc(
        "TRN2", target_bir_lowering=False, debug=False, num_devices=N_CORES
    )
    vis = nc.dram_tensor("vis", [B_PER_CORE, C, HWPIX], d, kind="ExternalInput")
    ir = nc.dram_tensor("ir", [B_PER_CORE, C, HWPIX], d, kind="ExternalInput")
    w1t = nc.dram_tensor("w1t", [KO, 128, HID1], d, kind="ExternalInput")
    b1 = nc.dram_tensor("b1", [HID1, 1], F32, kind="ExternalInput")
    w2t = nc.dram_tensor("w2t", [HID1, C], d, kind="ExternalInput")
    iden = nc.dram_tensor("iden", [128, 4 * 128], d, kind="ExternalInput")
    out = nc.dram_tensor("out", [B_PER_CORE, C, HWPIX], F32, kind="ExternalOutput")

    big = variant in ("big", "v4", "v5", "v5t")
    ko_outer = variant == "v4"
    halves = variant in ("v5", "v5t", "v6")
    v6 = variant == "v6"
    tweak_ends = variant == "v5t"
    last_bj = (B_PER_CORE - 1, HWPIX // NBIG - 1)
    with TileContext(nc) as tc:
        with (
            tc.tile_pool(name="consts", bufs=1) as cpool,
            tc.tile_pool(name="inbuf", bufs=INBUFS) as inpool,
            tc.tile_pool(name="work", bufs=2) as wpool,
            tc.tile_pool(name="outbuf", bufs=2) as opool,
            tc.tile_pool(
                name="ps1",
                bufs=int(os.environ.get("KPS1", "4" if big else "3")),
                space="PSUM",
            ) as ps1pool,
            tc.tile_pool(
                name="ps2", bufs=int(os.environ.get("KPS2", "2")), space="PSUM"
            ) as ps2pool,
        ):
            # consts ride the ACT HWDGE queue in v5 so the SP queue's very
            # first entries are real input data
            cdma = nc.scalar if halves else nc.sync
            # iden first: it feeds the PE warm-up, so it must land as early
            # as the engine preamble allows
            iden_sb = cpool.tile([128, 4 * 128], d, tag="iden")
            cdma.dma_start(iden_sb, iden[:, :])
            w1t_sb = cpool.tile([128, KO, HID1], d, tag="w1t")
            cdma.dma_start(w1t_sb, w1t[:, :, :].rearrange("ko p m -> p ko m"))
            w2t_sb = cpool.tile([HID1, C], d, tag="w2t")
            cdma.dma_start(w2t_sb, w2t[:, :])
            b1_sb = cpool.tile([HID1, 1], F32, tag="b1")
            cdma.dma_start(b1_sb, b1[:, :])

            # PE warm-up: HAM throttles the PE to 1.2 GHz until it sees
            # ~3.4us of sustained matmul activity. Run dummy matmuls on
            # const data during the startup DMA window so the real work
            # starts (and stays) at 2.4 GHz. All into one PSUM tile
            # (same-bank WAW keeps them back-to-back on the PE, and a
            # single allocation avoids pool-rotation stalls). Results are
            # never read.
            n_warm = int(os.environ.get("KWARM", "12"))
            if n_warm:
                warm_ps = ps2pool.tile([128, 2 * NT], F32, tag="ps2", name="warm_ps")
                for w in range(n_warm):
                    nc.tensor.matmul(
                        warm_ps[:, :NT],
                        lhsT=iden_sb[:, :128],
                        rhs=iden_sb[:, :NT],
                        start=True,
                        stop=True,
                    )

            def supertile_v6(b, j):
                """Per-half software pipeline: DMA(h) -> bypass-add(h) +
                first-layer(h) -> second-layer(h) -> out-DMA(h). The tail
                of the whole kernel is just one half-chunk's phase B."""
                ins = {}
                for nm in ("v", "i"):
                    for c in range(CCH):
                        ins[(nm, c)] = inpool.tile(
                            [128, NBIG], d, tag=f"in_{nm}{c}", name=f"in_{nm}{c}"
                        )
                outs = [
                    opool.tile([128, NBIG], F32, tag=f"out{c}", name=f"outt{c}")
                    for c in range(CCH)
                ]
                x_big = wpool.tile([HID1, NBIG], d, tag="x", name="x_big")
                rhs_order = [("v", 0), ("v", 1), ("i", 0), ("i", 1)]
                for h in range(2):
                    hw_ = NBIG // 2
                    hs = slice(h * hw_, (h + 1) * hw_)
                    gs = slice(j * NBIG + h * hw_, j * NBIG + (h + 1) * hw_)
                    for nm, dram in (("v", vis), ("i", ir)):
                        for c in range(CCH):
                            nc.sync.dma_start(
                                ins[(nm, c)][:, hs],
                                dram[b, c * 128 : (c + 1) * 128, gs],
                            )
                    for c in range(CCH):
                        nc.vector.tensor_add(
                            outs[c][:, hs],
                            ins[("v", c)][:, hs].bitcast(F32),
                            ins[("i", c)][:, hs].bitcast(F32),
                        )
                    for js in (2 * h, 2 * h + 1):
                        sl = slice(js * NT, (js + 1) * NT)
                        ps1 = ps1pool.tile([HID1, NT], F32, tag="ps1", name="ps1")
                        for ko, key in enumerate(rhs_order):
                            nc.tensor.matmul(
                                ps1,
                                lhsT=w1t_sb[:, ko],
                                rhs=ins[key][:, sl],
                                start=(ko == 0),
                                stop=(ko == KO - 1),
                            )
                        nc.scalar.activation(
                            x_big[:, sl], ps1,
                            mybir.ActivationFunctionType.Relu,
                            bias=b1_sb[:, 0:1],
                        )
                    for c in range(CCH):
                        ps2 = ps2pool.tile([128, 2 * NT], F32, tag="ps2", name="ps2")
                        for q in range(2):
                            xsl = slice((2 * h + q) * NT, (2 * h + q + 1) * NT)
                            nc.tensor.matmul(
                                ps2[:, q * NT : (q + 1) * NT],
                                lhsT=w2t_sb[:, c * 128 : (c + 1) * 128],
                                rhs=x_big[:, xsl],
                                start=True,
                                stop=True,
                            )
                        nc.vector.tensor_add(outs[c][:, hs], outs[c][:, hs], ps2)
                        nc.scalar.dma_start(
                            out[b, c * 128 : (c + 1) * 128, gs], outs[c][:, hs]
                        )

            for b in range(B_PER_CORE):
                for j in range(HWPIX // NBIG):
                    if v6:
                        supertile_v6(b, j)
                        continue
                    jsl = slice(j * NBIG, (j + 1) * NBIG)
                    ins = {}
                    if halves:
                        # interleaved half-tile DMAs: the single SP queue is
                        # FIFO, so this ordering lands a complete half-set
                        # (all 4 inputs' columns h) every ~5.6us instead of
                        # one whole input every ~2.8us. PE can start each
                        # js-pair as its half-set arrives; idle slivers stay
                        # under the ~3.4us HAM window.
                        for nm, dram in (("v", vis), ("i", ir)):
                            for c in range(CCH):
                                ins[(nm, c)] = inpool.tile(
                                    [128, NBIG], d,
                                    tag=f"in_{nm}{c}", name=f"in_{nm}{c}",
                                )
                        # the very first supertile splits its input halves
                        # across both HWDGE queues (outputs don't need the
                        # ACT queue yet) so the first half-set lands in
                        # ~2.8us instead of ~5.6us
                        dual = tweak_ends and b == 0 and j == 0
                        for h in range(2):
                            hs = slice(h * (NBIG // 2), (h + 1) * (NBIG // 2))
                            gs = slice(
                                j * NBIG + h * (NBIG // 2),
                                j * NBIG + (h + 1) * (NBIG // 2),
                            )
                            for nm, dram in (("v", vis), ("i", ir)):
                                eng = nc.scalar if (dual and nm == "i") else nc.sync
                                for c in range(CCH):
                                    eng.dma_start(
                                        ins[(nm, c)][:, hs],
                                        dram[b, c * 128 : (c + 1) * 128, gs],
                                    )
                    else:
                        for nm, dram in (("v", vis), ("i", ir)):
                            for c in range(CCH):
                                t = inpool.tile(
                                    [128, NBIG], d,
                                    tag=f"in_{nm}{c}", name=f"in_{nm}{c}",
                                )
                                nc.sync.dma_start(
                                    t, dram[b, c * 128 : (c + 1) * 128, jsl]
                                )
                                ins[(nm, c)] = t
                    outs = [
                        opool.tile([128, NBIG], F32, tag=f"out{c}", name=f"outt{c}")
                        for c in range(CCH)
                    ]
                    if big:
                        # bypass sum first: outs[c] = vis_c + ir_c in big DVE
                        # ops (fewer instructions, runs while PE does phase A)
                        for c in range(CCH):
                            if halves:
                                for h in range(2):
                                    hs = slice(
                                        h * (NBIG // 2), (h + 1) * (NBIG // 2)
                                    )
                                    nc.vector.tensor_add(
                                        outs[c][:, hs],
                                        ins[("v", c)][:, hs].bitcast(F32),
                                        ins[("i", c)][:, hs].bitcast(F32),
                                    )
                            else:
                                nc.vector.tensor_add(
                                    outs[c],
                                    ins[("v", c)].bitcast(F32),
                                    ins[("i", c)].bitcast(F32),
                                )
                    # Phase A: first layer for the whole supertile.
                    # x rows 0..31 = relu(W1@c + b1); row 32 = relu(0+1) = 1
                    x_big = wpool.tile([HID1, NBIG], d, tag="x", name="x_big")
                    rhs_order = [("v", 0), ("v", 1), ("i", 0), ("i", 1)]
                    if ko_outer:
                        # K-outer: the single HWDGE queue delivers the four
                        # input tiles sequentially (~2.8us apart), so consume
                        # each K chunk as it lands instead of waiting for all
                        # four. Keeps PE idle slivers under the ~3.4us HAM
                        # window -> matmuls stay at 2.4 GHz.
                        ps1s = [
                            ps1pool.tile([HID1, NT], F32, tag="ps1", name="ps1")
                            for _ in range(NBIG // NT)
                        ]
                        for ko, key in enumerate(rhs_order):
                            for js in range(NBIG // NT):
                                sl = slice(js * NT, (js + 1) * NT)
                                nc.tensor.matmul(
                                    ps1s[js],
                                    lhsT=w1t_sb[:, ko],
                                    rhs=ins[key][:, sl],
                                    start=(ko == 0),
                                    stop=(ko == KO - 1),
                                )
                        for js in range(NBIG // NT):
                            sl = slice(js * NT, (js + 1) * NT)
                            nc.scalar.activation(
                                x_big[:, sl], ps1s[js],
                                mybir.ActivationFunctionType.Relu,
                                bias=b1_sb[:, 0:1],
                            )
                    else:
                        for js in range(NBIG // NT):
                            sl = slice(js * NT, (js + 1) * NT)
                            ps1 = ps1pool.tile([HID1, NT], F32, tag="ps1", name="ps1")
                            for ko, key in enumerate(rhs_order):
                                nc.tensor.matmul(
                                    ps1,
                                    lhsT=w1t_sb[:, ko],
                                    rhs=ins[key][:, sl],
                                    start=(ko == 0),
                                    stop=(ko == KO - 1),
                                )
                            nc.scalar.activation(
                                x_big[:, sl], ps1, mybir.ActivationFunctionType.Relu,
                                bias=b1_sb[:, 0:1],
                            )
                    # Phase B: second layer + bypass adds.
                    if tweak_ends and (b, j) == last_bj:
                        # final supertile: NT-granular phase B so the kernel
                        # tail is one small chunk's matmul+add+DMA chain
                        for js in range(NBIG // NT):
                            sl = slice(js * NT, (js + 1) * NT)
                            for c in range(CCH):
                                ps2 = ps2pool.tile(
                                    [128, 2 * NT], F32, tag="ps2", name="ps2"
                                )
                                nc.tensor.matmul(
                                    ps2[:, :NT],
                                    lhsT=w2t_sb[:, c * 128 : (c + 1) * 128],
                                    rhs=x_big[:, sl],
                                    start=True,
                                    stop=True,
                                )
                                nc.vector.tensor_add(
                                    outs[c][:, sl], outs[c][:, sl], ps2[:, :NT]
                                )
                                nc.scalar.dma_start(
                                    out[
                                        b,
                                        c * 128 : (c + 1) * 128,
                                        j * NBIG + js * NT : j * NBIG
                                        + (js + 1) * NT,
                                    ],
                                    outs[c][:, sl],
                                )
                        continue
                    if big:
                        # mm2 in [128, 2*NT] psum chunks; one in-place DVE
                        # add per chunk (outs += residual+b2), then DMA the
                        # finished half straight out.
                        for h in range(NBIG // (2 * NT)):
                            hsl = slice(h * 2 * NT, (h + 1) * 2 * NT)
                            for c in range(CCH):
                                ps2 = ps2pool.tile(
                                    [128, 2 * NT], F32, tag="ps2", name="ps2"
                                )
                                for q in range(2):
                                    xsl = slice(
                                        (2 * h + q) * NT, (2 * h + q + 1) * NT
                                    )
                                    nc.tensor.matmul(
                                        ps2[:, q * NT : (q + 1) * NT],
                                        lhsT=w2t_sb[:, c * 128 : (c + 1) * 128],
                                        rhs=x_big[:, xsl],
                                        start=True,
                                        stop=True,
                                    )
                                nc.vector.tensor_add(
                                    outs[c][:, hsl], outs[c][:, hsl], ps2
                                )
                                # out-DMAs ride the ACT HWDGE queue so they
                                # never delay input prefetch on the SP queue
                                nc.scalar.dma_start(
                                    out[
                                        b,
                                        c * 128 : (c + 1) * 128,
                                        j * NBIG + h * 2 * NT : j * NBIG
                                        + (h + 1) * 2 * NT,
                                    ],
                                    outs[c][:, hsl],
                                )
                        continue
                    for js in range(NBIG // NT):
                        sl = slice(js * NT, (js + 1) * NT)
                        for c in range(CCH):
                            ps2 = ps2pool.tile(
                                [128, NT], F32, tag=f"ps2_{c}", name=f"ps2_{c}"
                            )
                            if hybrid:
                                # psum2 = I.T @ ir_c  (ir bypass on the PE)
                                nc.tensor.matmul(
                                    ps2,
                                    lhsT=iden_sb[:, :128],
                                    rhs=ins[("i", c)][:, sl],
                                    start=True,
                                    stop=False,
                                )
                            # psum2 += [W2; b2].T @ [x; 1] = residual + b2
                            nc.tensor.matmul(
                                ps2,
                                lhsT=w2t_sb[:, c * 128 : (c + 1) * 128],
                                rhs=x_big[:, sl],
                                start=not hybrid,
                                stop=True,
                            )
                            if hybrid:
                                nc.vector.tensor_add(
                                    outs[c][:, sl],
                                    ps2,
                                    ins[("v", c)][:, sl].bitcast(F32),
                                )
                            else:
                                s_t = wpool.tile([128, NT], F32, tag="s", name="s_t")
                                nc.vector.tensor_add(
                                    s_t,
                                    ins[("v", c)][:, sl].bitcast(F32),
                                    ins[("i", c)][:, sl].bitcast(F32),
                                )
                                nc.vector.tensor_add(outs[c][:, sl], ps2, s_t)
                    for c in range(CCH):
                        nc.sync.dma_start(
                            out[b, c * 128 : (c + 1) * 128, jsl], outs[c]
                        )
    nc.compile()
    return nc


def _get_nc(mm_dt: str, variant: str) -> bass.Bass:
    key = ("nc", mm_dt, variant)
    if key not in _cache:
        _cache[key] = _build(mm_dt, variant)
    return _cache[key]


def kernel(
    visible_features: np.ndarray,
    infrared_features: np.ndarray,
    W1: np.ndarray,
    b1: np.ndarray,
    W2: np.ndarray,
    b2: np.ndarray,
    _mm_dt: str = "f32r",
    _variant: str = "v5",
    _trace: bool = False,
) -> np.ndarray:
    nc = _get_nc(_mm_dt, _variant)

    vis = np.ascontiguousarray(visible_features, dtype=np.float32).reshape(B, C, HWPIX)
    ir = np.ascontiguousarray(infrared_features, dtype=np.float32).reshape(B, C, HWPIX)

    w1t = np.zeros((2 * C, HID1), dtype=np.float32)
    w1t[:, :HID] = W1.astype(np.float32).T
    w1t = np.ascontiguousarray(w1t.reshape(KO, 128, HID1))
    b1r = np.ones((HID1, 1), dtype=np.float32)
    b1r[:HID, 0] = b1.astype(np.float32)
    w2t = np.zeros((HID1, C), dtype=np.float32)
    w2t[:HID] = W2.astype(np.float32).T
    w2t[HID] = b2.astype(np.float32)
    iden = np.zeros((128, 4 * 128), dtype=np.float32)
    iden[:, :128] = np.eye(128, dtype=np.float32)

    in_maps = []
    for core in range(N_CORES):
        bsl = slice(core * B_PER_CORE, (core + 1) * B_PER_CORE)
        in_maps.append(
            {
                "vis": vis[bsl],
                "ir": ir[bsl],
                "w1t": w1t,
                "b1": b1r,
                "w2t": w2t,
                "iden": iden,
            }
        )

    res = bass_utils.run_bass_kernel_spmd(
        nc, in_maps, core_ids=list(range(N_CORES)), trace=_trace
    )
    if _trace:
        kernel.last_results = res
    outs = [r["out"] for r in res.results]
    return np.concatenate(outs, axis=0).reshape(B, C, H, W)



# revision 8
# speedup vs baseline: 1.3741x; 1.3741x over previous
"""CrossModalFeatureInteraction kernel for Trainium2 (Bass/Tile), 8 NeuronCores.

Computation (per pixel, per batch):
    combined = concat([vis, ir], channel)              # [512]
    x        = relu(W1 @ combined + b1)                # [32]
    residual = W2 @ x + b2                             # [256]
    out      = vis + ir + residual                     # [256]

Sharding: data-parallel over batch. B=16 -> 2 images per core on 8 cores.
Weights are tiny and replicated. Each core streams its 2 images through
SBUF in pixel supertiles; 1x1 convs are matmuls with channels as the
contraction dim and pixels as the moving free dim.

Engine budget tricks (target regime is memory; DMA ~77us/core is the
roofline, so every other engine must stay well under it):
  - Matmuls run in float32r: full-rate (1 col/cycle) PE mode on fp32 bits.
  - b1 rides as the activation bias; an all-zero 33rd W1 column plus
    bias=1.0 makes x's 33rd row == 1.0, so b2 rides as the 33rd row of
    W2 (K=33 second matmul). No separate bias pass.
  - Each supertile runs in two phases: all first-layer matmuls + relus
    into one batched x tile first, then all second-layer matmuls. The
    PE never waits on the ACT relu this way (its consumer runs ~4 tile
    slots behind the producer), so it stays dense and HAM-warm.
  - variant "hybrid": the ir half of the bypass is added by the PE (an
    identity matmul accumulating into the residual PSUM bank), leaving
    DVE one add per output tile. variant "dve": both bypass adds on DVE
    (fp32 tensor_tensor is 1x mode, so this doubles DVE time but frees
    the PE).
"""

import numpy as np

import concourse.bass as bass
import concourse.mybir as mybir
from concourse import bacc, bass_utils
from concourse.tile import TileContext

# Problem shape (hardcoded per contract)
B, C, H, W = 16, 256, 64, 64
HID = 32
HWPIX = H * W          # 4096 pixels per image
N_CORES = 8
B_PER_CORE = B // N_CORES  # 2

import os
NBIG = int(os.environ.get("KNBIG", "2048"))  # pixels per DMA supertile (1 MiB tiles)
INBUFS = int(os.environ.get("KINBUFS", "4"))  # input-tile double-buffer depth
NT = 512               # matmul moving free dim (one PSUM bank of fp32)
KO = 4                 # 512 combined channels / 128 partitions
CCH = 2                # 256 output channels / 128 partitions
HID1 = HID + 1         # hidden + ones row (carries b2 through matmul 2)

F32 = mybir.dt.float32
F32R = mybir.dt.float32r
BF16 = mybir.dt.bfloat16

_cache = {}


def _build(mm_dt: str, variant: str) -> bass.Bass:
    if mm_dt == "bf16":
        d = BF16
    elif mm_dt == "f32r":
        d = F32R
    else:
        d = F32
    bf16 = d is BF16
    # IO dtype for the output tensor / out tiles: bf16 halves store traffic
    # (rel-err budget is 2e-2; all-bf16 IO lands at ~2.4e-3).
    od = BF16 if bf16 else F32
    hybrid = variant == "hybrid"

    nc = bacc.Bacc(
        "TRN2", target_bir_lowering=False, debug=False, num_devices=N_CORES
    )
    vis = nc.dram_tensor("vis", [B_PER_CORE, C, HWPIX], d, kind="ExternalInput")
    ir = nc.dram_tensor("ir", [B_PER_CORE, C, HWPIX], d, kind="ExternalInput")
    w1t = nc.dram_tensor("w1t", [KO, 128, HID1], d, kind="ExternalInput")
    b1 = nc.dram_tensor("b1", [HID1, 1], F32, kind="ExternalInput")
    w2t = nc.dram_tensor("w2t", [HID1, C], d, kind="ExternalInput")
    iden = nc.dram_tensor("iden", [128, 4 * 128], d, kind="ExternalInput")
    out = nc.dram_tensor("out", [B_PER_CORE, C, HWPIX], od, kind="ExternalOutput")

    # DVE operand view: f32r tiles must be bitcast to f32; bf16 tiles are
    # consumed natively (16-bit DVE ops run at 2x).
    asf = (lambda t: t) if bf16 else (lambda t: t.bitcast(F32))

    big = variant in ("big", "v4", "v5", "v5t")
    ko_outer = variant == "v4"
    halves = variant in ("v5", "v5t", "v6")
    v6 = variant == "v6"
    tweak_ends = variant == "v5t"
    last_bj = (B_PER_CORE - 1, HWPIX // NBIG - 1)
    with TileContext(nc) as tc:
        with (
            tc.tile_pool(name="consts", bufs=1) as cpool,
            tc.tile_pool(name="inbuf", bufs=INBUFS) as inpool,
            tc.tile_pool(name="work", bufs=2) as wpool,
            tc.tile_pool(name="outbuf", bufs=2) as opool,
            tc.tile_pool(
                name="ps1",
                bufs=int(os.environ.get("KPS1", "4" if big else "3")),
                space="PSUM",
            ) as ps1pool,
            tc.tile_pool(
                name="ps2", bufs=int(os.environ.get("KPS2", "2")), space="PSUM"
            ) as ps2pool,
        ):
            # consts ride the ACT HWDGE queue in v5 so the SP queue's very
            # first entries are real input data
            cdma = nc.scalar if halves else nc.sync
            # iden first: it feeds the PE warm-up, so it must land as early
            # as the engine preamble allows
            iden_sb = cpool.tile([128, 4 * 128], d, tag="iden")
            cdma.dma_start(iden_sb, iden[:, :])
            w1t_sb = cpool.tile([128, KO, HID1], d, tag="w1t")
            cdma.dma_start(w1t_sb, w1t[:, :, :].rearrange("ko p m -> p ko m"))
            w2t_sb = cpool.tile([HID1, C], d, tag="w2t")
            cdma.dma_start(w2t_sb, w2t[:, :])
            b1_sb = cpool.tile([HID1, 1], F32, tag="b1")
            cdma.dma_start(b1_sb, b1[:, :])

            # PE warm-up: HAM throttles the PE to 1.2 GHz until it sees
            # ~3.4us of sustained matmul activity. Run dummy matmuls on
            # const data during the startup DMA window so the real work
            # starts (and stays) at 2.4 GHz. All into one PSUM tile
            # (same-bank WAW keeps them back-to-back on the PE, and a
            # single allocation avoids pool-rotation stalls). Results are
            # never read.
            n_warm = int(os.environ.get("KWARM", "12"))
            if n_warm:
                warm_ps = ps2pool.tile([128, 2 * NT], F32, tag="ps2", name="warm_ps")
                for w in range(n_warm):
                    nc.tensor.matmul(
                        warm_ps[:, :NT],
                        lhsT=iden_sb[:, :128],
                        rhs=iden_sb[:, :NT],
                        start=True,
                        stop=True,
                    )

            def supertile_v6(b, j):
                """Per-half software pipeline: DMA(h) -> bypass-add(h) +
                first-layer(h) -> second-layer(h) -> out-DMA(h). The tail
                of the whole kernel is just one half-chunk's phase B."""
                ins = {}
                for nm in ("v", "i"):
                    for c in range(CCH):
                        ins[(nm, c)] = inpool.tile(
                            [128, NBIG], d, tag=f"in_{nm}{c}", name=f"in_{nm}{c}"
                        )
                outs = [
                    opool.tile([128, NBIG], od, tag=f"out{c}", name=f"outt{c}")
                    for c in range(CCH)
                ]
                x_big = wpool.tile([HID1, NBIG], d, tag="x", name="x_big")
                rhs_order = [("v", 0), ("v", 1), ("i", 0), ("i", 1)]
                for h in range(2):
                    hw_ = NBIG // 2
                    hs = slice(h * hw_, (h + 1) * hw_)
                    gs = slice(j * NBIG + h * hw_, j * NBIG + (h + 1) * hw_)
                    for nm, dram in (("v", vis), ("i", ir)):
                        for c in range(CCH):
                            nc.sync.dma_start(
                                ins[(nm, c)][:, hs],
                                dram[b, c * 128 : (c + 1) * 128, gs],
                            )
                    for c in range(CCH):
                        nc.vector.tensor_add(
                            outs[c][:, hs],
                            asf(ins[("v", c)][:, hs]),
                            asf(ins[("i", c)][:, hs]),
                        )
                    for js in (2 * h, 2 * h + 1):
                        sl = slice(js * NT, (js + 1) * NT)
                        ps1 = ps1pool.tile([HID1, NT], F32, tag="ps1", name="ps1")
                        for ko, key in enumerate(rhs_order):
                            nc.tensor.matmul(
                                ps1,
                                lhsT=w1t_sb[:, ko],
                                rhs=ins[key][:, sl],
                                start=(ko == 0),
                                stop=(ko == KO - 1),
                            )
                        nc.scalar.activation(
                            x_big[:, sl], ps1,
                            mybir.ActivationFunctionType.Relu,
                            bias=b1_sb[:, 0:1],
                        )
                    for c in range(CCH):
                        ps2 = ps2pool.tile([128, 2 * NT], F32, tag="ps2", name="ps2")
                        for q in range(2):
                            xsl = slice((2 * h + q) * NT, (2 * h + q + 1) * NT)
                            nc.tensor.matmul(
                                ps2[:, q * NT : (q + 1) * NT],
                                lhsT=w2t_sb[:, c * 128 : (c + 1) * 128],
                                rhs=x_big[:, xsl],
                                start=True,
                                stop=True,
                            )
                        nc.vector.tensor_add(outs[c][:, hs], outs[c][:, hs], ps2)
                        nc.scalar.dma_start(
                            out[b, c * 128 : (c + 1) * 128, gs], outs[c][:, hs]
                        )

            for b in range(B_PER_CORE):
                for j in range(HWPIX // NBIG):
                    if v6:
                        supertile_v6(b, j)
                        continue
                    jsl = slice(j * NBIG, (j + 1) * NBIG)
                    ins = {}
                    if halves:
                        # interleaved half-tile DMAs: the single SP queue is
                        # FIFO, so this ordering lands a complete half-set
                        # (all 4 inputs' columns h) every ~5.6us instead of
                        # one whole input every ~2.8us. PE can start each
                        # js-pair as its half-set arrives; idle slivers stay
                        # under the ~3.4us HAM window.
                        for nm, dram in (("v", vis), ("i", ir)):
                            for c in range(CCH):
                                ins[(nm, c)] = inpool.tile(
                                    [128, NBIG], d,
                                    tag=f"in_{nm}{c}", name=f"in_{nm}{c}",
                                )
                        # the very first supertile splits its input halves
                        # across both HWDGE queues (outputs don't need the
                        # ACT queue yet) so the first half-set lands in
                        # ~2.8us instead of ~5.6us
                        dual = tweak_ends and b == 0 and j == 0
                        for h in range(2):
                            hs = slice(h * (NBIG // 2), (h + 1) * (NBIG // 2))
                            gs = slice(
                                j * NBIG + h * (NBIG // 2),
                                j * NBIG + (h + 1) * (NBIG // 2),
                            )
                            for nm, dram in (("v", vis), ("i", ir)):
                                eng = nc.scalar if (dual and nm == "i") else nc.sync
                                for c in range(CCH):
                                    eng.dma_start(
                                        ins[(nm, c)][:, hs],
                                        dram[b, c * 128 : (c + 1) * 128, gs],
                                    )
                    else:
                        for nm, dram in (("v", vis), ("i", ir)):
                            for c in range(CCH):
                                t = inpool.tile(
                                    [128, NBIG], d,
                                    tag=f"in_{nm}{c}", name=f"in_{nm}{c}",
                                )
                                nc.sync.dma_start(
                                    t, dram[b, c * 128 : (c + 1) * 128, jsl]
                                )
                                ins[(nm, c)] = t
                    outs = [
                        opool.tile([128, NBIG], od, tag=f"out{c}", name=f"outt{c}")
                        for c in range(CCH)
                    ]
                    if big:
                        # bypass sum first: outs[c] = vis_c + ir_c in big DVE
                        # ops (fewer instructions, runs while PE does phase A)
                        for c in range(CCH):
                            if halves:
                                for h in range(2):
                                    hs = slice(
                                        h * (NBIG // 2), (h + 1) * (NBIG // 2)
                                    )
                                    nc.vector.tensor_add(
                                        outs[c][:, hs],
                                        asf(ins[("v", c)][:, hs]),
                                        asf(ins[("i", c)][:, hs]),
                                    )
                            else:
                                nc.vector.tensor_add(
                                    outs[c],
                                    asf(ins[("v", c)]),
                                    asf(ins[("i", c)]),
                                )
                    # Phase A: first layer for the whole supertile.
                    # x rows 0..31 = relu(W1@c + b1); row 32 = relu(0+1) = 1
                    x_big = wpool.tile([HID1, NBIG], d, tag="x", name="x_big")
                    rhs_order = [("v", 0), ("v", 1), ("i", 0), ("i", 1)]
                    if ko_outer:
                        # K-outer: the single HWDGE queue delivers the four
                        # input tiles sequentially (~2.8us apart), so consume
                        # each K chunk as it lands instead of waiting for all
                        # four. Keeps PE idle slivers under the ~3.4us HAM
                        # window -> matmuls stay at 2.4 GHz.
                        ps1s = [
                            ps1pool.tile([HID1, NT], F32, tag="ps1", name="ps1")
                            for _ in range(NBIG // NT)
                        ]
                        for ko, key in enumerate(rhs_order):
                            for js in range(NBIG // NT):
                                sl = slice(js * NT, (js + 1) * NT)
                                nc.tensor.matmul(
                                    ps1s[js],
                                    lhsT=w1t_sb[:, ko],
                                    rhs=ins[key][:, sl],
                                    start=(ko == 0),
                                    stop=(ko == KO - 1),
                                )
                        for js in range(NBIG // NT):
                            sl = slice(js * NT, (js + 1) * NT)
                            nc.scalar.activation(
                                x_big[:, sl], ps1s[js],
                                mybir.ActivationFunctionType.Relu,
                                bias=b1_sb[:, 0:1],
                            )
                    else:
                        for js in range(NBIG // NT):
                            sl = slice(js * NT, (js + 1) * NT)
                            ps1 = ps1pool.tile([HID1, NT], F32, tag="ps1", name="ps1")
                            for ko, key in enumerate(rhs_order):
                                nc.tensor.matmul(
                                    ps1,
                                    lhsT=w1t_sb[:, ko],
                                    rhs=ins[key][:, sl],
                                    start=(ko == 0),
                                    stop=(ko == KO - 1),
                                )
                            nc.scalar.activation(
                                x_big[:, sl], ps1, mybir.ActivationFunctionType.Relu,
                                bias=b1_sb[:, 0:1],
                            )
                    # Phase B: second layer + bypass adds.
                    if tweak_ends and (b, j) == last_bj:
                        # final supertile: NT-granular phase B so the kernel
                        # tail is one small chunk's matmul+add+DMA chain
                        for js in range(NBIG // NT):
                            sl = slice(js * NT, (js + 1) * NT)
                            for c in range(CCH):
                                ps2 = ps2pool.tile(
                                    [128, 2 * NT], F32, tag="ps2", name="ps2"
                                )
                                nc.tensor.matmul(
                                    ps2[:, :NT],
                                    lhsT=w2t_sb[:, c * 128 : (c + 1) * 128],
                                    rhs=x_big[:, sl],
                                    start=True,
                                    stop=True,
                                )
                                nc.vector.tensor_add(
                                    outs[c][:, sl], outs[c][:, sl], ps2[:, :NT]
                                )
                                nc.scalar.dma_start(
                                    out[
                                        b,
                                        c * 128 : (c + 1) * 128,
                                        j * NBIG + js * NT : j * NBIG
                                        + (js + 1) * NT,
                                    ],
                                    outs[c][:, sl],
                                )
                        continue
                    if big:
                        # mm2 in [128, 2*NT] psum chunks; one in-place DVE
                        # add per chunk (outs += residual+b2), then DMA the
                        # finished half straight out.
                        for h in range(NBIG // (2 * NT)):
                            hsl = slice(h * 2 * NT, (h + 1) * 2 * NT)
                            for c in range(CCH):
                                ps2 = ps2pool.tile(
                                    [128, 2 * NT], F32, tag="ps2", name="ps2"
                                )
                                for q in range(2):
                                    xsl = slice(
                                        (2 * h + q) * NT, (2 * h + q + 1) * NT
                                    )
                                    nc.tensor.matmul(
                                        ps2[:, q * NT : (q + 1) * NT],
                                        lhsT=w2t_sb[:, c * 128 : (c + 1) * 128],
                                        rhs=x_big[:, xsl],
                                        start=True,
                                        stop=True,
                                    )
                                nc.vector.tensor_add(
                                    outs[c][:, hsl], outs[c][:, hsl], ps2
                                )
                                # out-DMAs ride the ACT HWDGE queue so they
                                # never delay input prefetch on the SP queue
                                nc.scalar.dma_start(
                                    out[
                                        b,
                                        c * 128 : (c + 1) * 128,
                                        j * NBIG + h * 2 * NT : j * NBIG
                                        + (h + 1) * 2 * NT,
                                    ],
                                    outs[c][:, hsl],
                                )
                        continue
                    for js in range(NBIG // NT):
                        sl = slice(js * NT, (js + 1) * NT)
                        for c in range(CCH):
                            ps2 = ps2pool.tile(
                                [128, NT], F32, tag=f"ps2_{c}", name=f"ps2_{c}"
                            )
                            if hybrid:
                                # psum2 = I.T @ ir_c  (ir bypass on the PE)
                                nc.tensor.matmul(
                                    ps2,
                                    lhsT=iden_sb[:, :128],
                                    rhs=ins[("i", c)][:, sl],
                                    start=True,
                                    stop=False,
                                )
                            # psum2 += [W2; b2].T @ [x; 1] = residual + b2
                            nc.tensor.matmul(
                                ps2,
                                lhsT=w2t_sb[:, c * 128 : (c + 1) * 128],
                                rhs=x_big[:, sl],
                                start=not hybrid,
                                stop=True,
                            )
                            if hybrid:
                                nc.vector.tensor_add(
                                    outs[c][:, sl],
                                    ps2,
                                    asf(ins[("v", c)][:, sl]),
                                )
                            else:
                                s_t = wpool.tile([128, NT], F32, tag="s", name="s_t")
                                nc.vector.tensor_add(
                                    s_t,
                                    asf(ins[("v", c)][:, sl]),
                                    asf(ins[("i", c)][:, sl]),
                                )
                                nc.vector.tensor_add(outs[c][:, sl], ps2, s_t)
                    for c in range(CCH):
                        nc.sync.dma_start(
                            out[b, c * 128 : (c + 1) * 128, jsl], outs[c]
                        )
    nc.compile()
    return nc


def _get_nc(mm_dt: str, variant: str) -> bass.Bass:
    key = ("nc", mm_dt, variant)
    if key not in _cache:
        _cache[key] = _build(mm_dt, variant)
    return _cache[key]


def kernel(
    visible_features: np.ndarray,
    infrared_features: np.ndarray,
    W1: np.ndarray,
    b1: np.ndarray,
    W2: np.ndarray,
    b2: np.ndarray,
    _mm_dt: str = "bf16",
    _variant: str = "v5",
    _trace: bool = False,
) -> np.ndarray:
    nc = _get_nc(_mm_dt, _variant)

    if _mm_dt == "bf16":
        import ml_dtypes

        io_np = ml_dtypes.bfloat16
    else:
        io_np = np.float32

    vis = np.ascontiguousarray(
        np.asarray(visible_features).astype(io_np).reshape(B, C, HWPIX)
    )
    ir = np.ascontiguousarray(
        np.asarray(infrared_features).astype(io_np).reshape(B, C, HWPIX)
    )

    w1t = np.zeros((2 * C, HID1), dtype=np.float32)
    w1t[:, :HID] = W1.astype(np.float32).T
    w1t = np.ascontiguousarray(w1t.reshape(KO, 128, HID1)).astype(io_np)
    b1r = np.ones((HID1, 1), dtype=np.float32)
    b1r[:HID, 0] = b1.astype(np.float32)
    w2t = np.zeros((HID1, C), dtype=np.float32)
    w2t[:HID] = W2.astype(np.float32).T
    w2t[HID] = b2.astype(np.float32)
    w2t = w2t.astype(io_np)
    iden = np.zeros((128, 4 * 128), dtype=np.float32)
    iden[:, :128] = np.eye(128, dtype=np.float32)
    iden = iden.astype(io_np)

    in_maps = []
    for core in range(N_CORES):
        bsl = slice(core * B_PER_CORE, (core + 1) * B_PER_CORE)
        in_maps.append(
            {
                "vis": vis[bsl],
                "ir": ir[bsl],
                "w1t": w1t,
                "b1": b1r,
                "w2t": w2t,
                "iden": iden,
            }
        )

    res = bass_utils.run_bass_kernel_spmd(
        nc, in_maps, core_ids=list(range(N_CORES)), trace=_trace
    )
    if _trace:
        kernel.last_results = res
    outs = [np.asarray(r["out"]).astype(np.float32) for r in res.results]
    return np.concatenate(outs, axis=0).reshape(B, C, H, W)



# revision 10
# speedup vs baseline: 1.5487x; 1.1271x over previous
"""CrossModalFeatureInteraction kernel for Trainium2 (Bass/Tile), 8 NeuronCores.

Computation (per pixel, per batch):
    combined = concat([vis, ir], channel)              # [512]
    x        = relu(W1 @ combined + b1)                # [32]
    residual = W2 @ x + b2                             # [256]
    out      = vis + ir + residual                     # [256]

Sharding: data-parallel over batch. B=16 -> 2 images per core on 8 cores.
Weights are tiny and replicated. Each core streams its 2 images through
SBUF in pixel supertiles; 1x1 convs are matmuls with channels as the
contraction dim and pixels as the moving free dim.

Engine budget tricks (target regime is memory; DMA ~77us/core is the
roofline, so every other engine must stay well under it):
  - Matmuls run in float32r: full-rate (1 col/cycle) PE mode on fp32 bits.
  - b1 rides as the activation bias; an all-zero 33rd W1 column plus
    bias=1.0 makes x's 33rd row == 1.0, so b2 rides as the 33rd row of
    W2 (K=33 second matmul). No separate bias pass.
  - Each supertile runs in two phases: all first-layer matmuls + relus
    into one batched x tile first, then all second-layer matmuls. The
    PE never waits on the ACT relu this way (its consumer runs ~4 tile
    slots behind the producer), so it stays dense and HAM-warm.
  - variant "hybrid": the ir half of the bypass is added by the PE (an
    identity matmul accumulating into the residual PSUM bank), leaving
    DVE one add per output tile. variant "dve": both bypass adds on DVE
    (fp32 tensor_tensor is 1x mode, so this doubles DVE time but frees
    the PE).
"""

import numpy as np

import concourse.bass as bass
import concourse.mybir as mybir
from concourse import bacc, bass_utils
from concourse.tile import TileContext

# Problem shape (hardcoded per contract)
B, C, H, W = 16, 256, 64, 64
HID = 32
HWPIX = H * W          # 4096 pixels per image
N_CORES = 8
B_PER_CORE = B // N_CORES  # 2

import os
NBIG = int(os.environ.get("KNBIG", "2048"))  # pixels per DMA supertile (1 MiB tiles)
INBUFS = int(os.environ.get("KINBUFS", "4"))  # input-tile double-buffer depth
NT = 512               # matmul moving free dim (one PSUM bank of fp32)
KO = 4                 # 512 combined channels / 128 partitions
CCH = 2                # 256 output channels / 128 partitions
HID1 = HID + 1         # hidden + ones row (carries b2 through matmul 2)

F32 = mybir.dt.float32
F32R = mybir.dt.float32r
BF16 = mybir.dt.bfloat16

_cache = {}


def _build(mm_dt: str, variant: str) -> bass.Bass:
    if mm_dt == "bf16":
        d = BF16
    elif mm_dt == "f32r":
        d = F32R
    else:
        d = F32
    bf16 = d is BF16
    # IO dtype for the output tensor / out tiles: bf16 halves store traffic
    # (rel-err budget is 2e-2; all-bf16 IO lands at ~2.4e-3).
    od = BF16 if bf16 else F32
    hybrid = variant == "hybrid"

    nc = bacc.Bacc(
        "TRN2", target_bir_lowering=False, debug=False, num_devices=N_CORES
    )
    vis = nc.dram_tensor("vis", [B_PER_CORE, C, HWPIX], d, kind="ExternalInput")
    ir = nc.dram_tensor("ir", [B_PER_CORE, C, HWPIX], d, kind="ExternalInput")
    w1t = nc.dram_tensor("w1t", [KO, 128, HID1], d, kind="ExternalInput")
    b1 = nc.dram_tensor("b1", [HID1, 1], F32, kind="ExternalInput")
    w2t = nc.dram_tensor("w2t", [HID1, C], d, kind="ExternalInput")
    iden = nc.dram_tensor("iden", [128, 4 * 128], d, kind="ExternalInput")
    out = nc.dram_tensor("out", [B_PER_CORE, C, HWPIX], od, kind="ExternalOutput")

    # DVE operand view: f32r tiles must be bitcast to f32; bf16 tiles are
    # consumed natively (16-bit DVE ops run at 2x).
    asf = (lambda t: t) if bf16 else (lambda t: t.bitcast(F32))

    big = variant in ("big", "v4", "v5", "v5t")
    ko_outer = variant == "v4"
    halves = variant in ("v5", "v5t", "v6")
    v6 = variant == "v6"
    tweak_ends = variant == "v5t"
    last_bj = (B_PER_CORE - 1, HWPIX // NBIG - 1)
    with TileContext(nc) as tc:
        with (
            tc.tile_pool(name="consts", bufs=1) as cpool,
            tc.tile_pool(name="inbuf", bufs=INBUFS) as inpool,
            tc.tile_pool(name="work", bufs=2) as wpool,
            tc.tile_pool(name="outbuf", bufs=2) as opool,
            tc.tile_pool(
                name="ps1",
                bufs=int(os.environ.get("KPS1", "4" if big else "3")),
                space="PSUM",
            ) as ps1pool,
            tc.tile_pool(
                name="ps2", bufs=int(os.environ.get("KPS2", "2")), space="PSUM"
            ) as ps2pool,
        ):
            # consts ride the ACT HWDGE queue in v5 so the SP queue's very
            # first entries are real input data
            cdma = nc.scalar if halves else nc.sync
            # iden first: it feeds the PE warm-up, so it must land as early
            # as the engine preamble allows
            iden_sb = cpool.tile([128, 4 * 128], d, tag="iden")
            cdma.dma_start(iden_sb, iden[:, :])
            w1t_sb = cpool.tile([128, KO, HID1], d, tag="w1t")
            cdma.dma_start(w1t_sb, w1t[:, :, :].rearrange("ko p m -> p ko m"))
            w2t_sb = cpool.tile([HID1, C], d, tag="w2t")
            cdma.dma_start(w2t_sb, w2t[:, :])
            b1_sb = cpool.tile([HID1, 1], F32, tag="b1")
            cdma.dma_start(b1_sb, b1[:, :])

            # PE warm-up: HAM throttles the PE to 1.2 GHz until it sees
            # ~3.4us of sustained matmul activity. Run dummy matmuls on
            # const data during the startup DMA window so the real work
            # starts (and stays) at 2.4 GHz. All into one PSUM tile
            # (same-bank WAW keeps them back-to-back on the PE, and a
            # single allocation avoids pool-rotation stalls). Results are
            # never read.
            n_warm = int(os.environ.get("KWARM", "12"))
            if n_warm:
                warm_ps = ps2pool.tile([128, 2 * NT], F32, tag="ps2", name="warm_ps")
                for w in range(n_warm):
                    nc.tensor.matmul(
                        warm_ps[:, :NT],
                        lhsT=iden_sb[:, :128],
                        rhs=iden_sb[:, :NT],
                        start=True,
                        stop=True,
                    )

            def supertile_v6(b, j):
                """Per-half software pipeline: DMA(h) -> bypass-add(h) +
                first-layer(h) -> second-layer(h) -> out-DMA(h). The tail
                of the whole kernel is just one half-chunk's phase B."""
                ins = {}
                for nm in ("v", "i"):
                    for c in range(CCH):
                        ins[(nm, c)] = inpool.tile(
                            [128, NBIG], d, tag=f"in_{nm}{c}", name=f"in_{nm}{c}"
                        )
                outs = [
                    opool.tile([128, NBIG], od, tag=f"out{c}", name=f"outt{c}")
                    for c in range(CCH)
                ]
                x_big = wpool.tile([HID1, NBIG], d, tag="x", name="x_big")
                rhs_order = [("v", 0), ("v", 1), ("i", 0), ("i", 1)]
                for h in range(2):
                    hw_ = NBIG // 2
                    hs = slice(h * hw_, (h + 1) * hw_)
                    gs = slice(j * NBIG + h * hw_, j * NBIG + (h + 1) * hw_)
                    for nm, dram in (("v", vis), ("i", ir)):
                        for c in range(CCH):
                            nc.sync.dma_start(
                                ins[(nm, c)][:, hs],
                                dram[b, c * 128 : (c + 1) * 128, gs],
                            )
                    for c in range(CCH):
                        nc.vector.tensor_add(
                            outs[c][:, hs],
                            asf(ins[("v", c)][:, hs]),
                            asf(ins[("i", c)][:, hs]),
                        )
                    for js in (2 * h, 2 * h + 1):
                        sl = slice(js * NT, (js + 1) * NT)
                        ps1 = ps1pool.tile([HID1, NT], F32, tag="ps1", name="ps1")
                        for ko, key in enumerate(rhs_order):
                            nc.tensor.matmul(
                                ps1,
                                lhsT=w1t_sb[:, ko],
                                rhs=ins[key][:, sl],
                                start=(ko == 0),
                                stop=(ko == KO - 1),
                            )
                        nc.scalar.activation(
                            x_big[:, sl], ps1,
                            mybir.ActivationFunctionType.Relu,
                            bias=b1_sb[:, 0:1],
                        )
                    for c in range(CCH):
                        ps2 = ps2pool.tile([128, 2 * NT], F32, tag="ps2", name="ps2")
                        for q in range(2):
                            xsl = slice((2 * h + q) * NT, (2 * h + q + 1) * NT)
                            nc.tensor.matmul(
                                ps2[:, q * NT : (q + 1) * NT],
                                lhsT=w2t_sb[:, c * 128 : (c + 1) * 128],
                                rhs=x_big[:, xsl],
                                start=True,
                                stop=True,
                            )
                        nc.vector.tensor_add(outs[c][:, hs], outs[c][:, hs], ps2)
                        nc.scalar.dma_start(
                            out[b, c * 128 : (c + 1) * 128, gs], outs[c][:, hs]
                        )

            for b in range(B_PER_CORE):
                for j in range(HWPIX // NBIG):
                    if v6:
                        supertile_v6(b, j)
                        continue
                    jsl = slice(j * NBIG, (j + 1) * NBIG)
                    ins = {}
                    if halves:
                        # interleaved half-tile DMAs: the single SP queue is
                        # FIFO, so this ordering lands a complete half-set
                        # (all 4 inputs' columns h) every ~5.6us instead of
                        # one whole input every ~2.8us. PE can start each
                        # js-pair as its half-set arrives; idle slivers stay
                        # under the ~3.4us HAM window.
                        for nm, dram in (("v", vis), ("i", ir)):
                            for c in range(CCH):
                                ins[(nm, c)] = inpool.tile(
                                    [128, NBIG], d,
                                    tag=f"in_{nm}{c}", name=f"in_{nm}{c}",
                                )
                        # the very first supertile splits its input halves
                        # across both HWDGE queues (outputs don't need the
                        # ACT queue yet) so the first half-set lands in
                        # ~2.8us instead of ~5.6us
                        dual = tweak_ends and b == 0 and j == 0
                        for h in range(2):
                            hs = slice(h * (NBIG // 2), (h + 1) * (NBIG // 2))
                            gs = slice(
                                j * NBIG + h * (NBIG // 2),
                                j * NBIG + (h + 1) * (NBIG // 2),
                            )
                            for nm, dram in (("v", vis), ("i", ir)):
                                eng = nc.scalar if (dual and nm == "i") else nc.sync
                                for c in range(CCH):
                                    eng.dma_start(
                                        ins[(nm, c)][:, hs],
                                        dram[b, c * 128 : (c + 1) * 128, gs],
                                    )
                    else:
                        for nm, dram in (("v", vis), ("i", ir)):
                            for c in range(CCH):
                                t = inpool.tile(
                                    [128, NBIG], d,
                                    tag=f"in_{nm}{c}", name=f"in_{nm}{c}",
                                )
                                nc.sync.dma_start(
                                    t, dram[b, c * 128 : (c + 1) * 128, jsl]
                                )
                                ins[(nm, c)] = t
                    outs = [
                        opool.tile([128, NBIG], od, tag=f"out{c}", name=f"outt{c}")
                        for c in range(CCH)
                    ]
                    if big:
                        # bypass sum first: outs[c] = vis_c + ir_c in big DVE
                        # ops (fewer instructions, runs while PE does phase A)
                        for c in range(CCH):
                            if halves:
                                for h in range(2):
                                    hs = slice(
                                        h * (NBIG // 2), (h + 1) * (NBIG // 2)
                                    )
                                    nc.vector.tensor_add(
                                        outs[c][:, hs],
                                        asf(ins[("v", c)][:, hs]),
                                        asf(ins[("i", c)][:, hs]),
                                    )
                            else:
                                nc.vector.tensor_add(
                                    outs[c],
                                    asf(ins[("v", c)]),
                                    asf(ins[("i", c)]),
                                )
                    # Phase A: first layer for the whole supertile.
                    # x rows 0..31 = relu(W1@c + b1); row 32 = relu(0+1) = 1
                    x_big = wpool.tile([HID1, NBIG], d, tag="x", name="x_big")
                    rhs_order = [("v", 0), ("v", 1), ("i", 0), ("i", 1)]
                    if ko_outer:
                        # K-outer: the single HWDGE queue delivers the four
                        # input tiles sequentially (~2.8us apart), so consume
                        # each K chunk as it lands instead of waiting for all
                        # four. Keeps PE idle slivers under the ~3.4us HAM
                        # window -> matmuls stay at 2.4 GHz.
                        ps1s = [
                            ps1pool.tile([HID1, NT], F32, tag="ps1", name="ps1")
                            for _ in range(NBIG // NT)
                        ]
                        for ko, key in enumerate(rhs_order):
                            for js in range(NBIG // NT):
                                sl = slice(js * NT, (js + 1) * NT)
                                nc.tensor.matmul(
                                    ps1s[js],
                                    lhsT=w1t_sb[:, ko],
                                    rhs=ins[key][:, sl],
                                    start=(ko == 0),
                                    stop=(ko == KO - 1),
                                )
                        for js in range(NBIG // NT):
                            sl = slice(js * NT, (js + 1) * NT)
                            nc.scalar.activation(
                                x_big[:, sl], ps1s[js],
                                mybir.ActivationFunctionType.Relu,
                                bias=b1_sb[:, 0:1],
                            )
                    else:
                        for js in range(NBIG // NT):
                            sl = slice(js * NT, (js + 1) * NT)
                            ps1 = ps1pool.tile([HID1, NT], F32, tag="ps1", name="ps1")
                            for ko, key in enumerate(rhs_order):
                                nc.tensor.matmul(
                                    ps1,
                                    lhsT=w1t_sb[:, ko],
                                    rhs=ins[key][:, sl],
                                    start=(ko == 0),
                                    stop=(ko == KO - 1),
                                )
                            nc.scalar.activation(
                                x_big[:, sl], ps1, mybir.ActivationFunctionType.Relu,
                                bias=b1_sb[:, 0:1],
                            )
                    # Phase B: second layer + bypass adds.
                    if tweak_ends and (b, j) == last_bj:
                        # final supertile: NT-granular phase B so the kernel
                        # tail is one small chunk's matmul+add+DMA chain
                        for js in range(NBIG // NT):
                            sl = slice(js * NT, (js + 1) * NT)
                            for c in range(CCH):
                                ps2 = ps2pool.tile(
                                    [128, 2 * NT], F32, tag="ps2", name="ps2"
                                )
                                nc.tensor.matmul(
                                    ps2[:, :NT],
                                    lhsT=w2t_sb[:, c * 128 : (c + 1) * 128],
                                    rhs=x_big[:, sl],
                                    start=True,
                                    stop=True,
                                )
                                nc.vector.tensor_add(
                                    outs[c][:, sl], outs[c][:, sl], ps2[:, :NT]
                                )
                                nc.scalar.dma_start(
                                    out[
                                        b,
                                        c * 128 : (c + 1) * 128,
                                        j * NBIG + js * NT : j * NBIG
                                        + (js + 1) * NT,
                                    ],
                                    outs[c][:, sl],
                                )
                        continue
                    if big:
                        # mm2 in [128, 2*NT] psum chunks; one in-place DVE
                        # add per chunk (outs += residual+b2), then DMA the
                        # finished half straight out.
                        for h in range(NBIG // (2 * NT)):
                            hsl = slice(h * 2 * NT, (h + 1) * 2 * NT)
                            for c in range(CCH):
                                ps2 = ps2pool.tile(
                                    [128, 2 * NT], F32, tag="ps2", name="ps2"
                                )
                                for q in range(2):
                                    xsl = slice(
                                        (2 * h + q) * NT, (2 * h + q + 1) * NT
                                    )
                                    nc.tensor.matmul(
                                        ps2[:, q * NT : (q + 1) * NT],
                                        lhsT=w2t_sb[:, c * 128 : (c + 1) * 128],
                                        rhs=x_big[:, xsl],
                                        start=True,
                                        stop=True,
                                    )
                                nc.vector.tensor_add(
                                    outs[c][:, hsl], outs[c][:, hsl], ps2
                                )
                                # out-DMAs ride the ACT HWDGE queue so they
                                # never delay input prefetch on the SP queue
                                nc.scalar.dma_start(
                                    out[
                                        b,
                                        c * 128 : (c + 1) * 128,
                                        j * NBIG + h * 2 * NT : j * NBIG
                                        + (h + 1) * 2 * NT,
                                    ],
                                    outs[c][:, hsl],
                                )
                        continue
                    for js in range(NBIG // NT):
                        sl = slice(js * NT, (js + 1) * NT)
                        for c in range(CCH):
                            ps2 = ps2pool.tile(
                                [128, NT], F32, tag=f"ps2_{c}", name=f"ps2_{c}"
                            )
                            if hybrid:
                                # psum2 = I.T @ ir_c  (ir bypass on the PE)
                                nc.tensor.matmul(
                                    ps2,
                                    lhsT=iden_sb[:, :128],
                                    rhs=ins[("i", c)][:, sl],
                                    start=True,
                                    stop=False,
                                )
                            # psum2 += [W2; b2].T @ [x; 1] = residual + b2
                            nc.tensor.matmul(
                                ps2,
                                lhsT=w2t_sb[:, c * 128 : (c + 1) * 128],
                                rhs=x_big[:, sl],
                                start=not hybrid,
                                stop=True,
                            )
                            if hybrid:
                                nc.vector.tensor_add(
                                    outs[c][:, sl],
                                    ps2,
                                    asf(ins[("v", c)][:, sl]),
                                )
                            else:
                                s_t = wpool.tile([128, NT], F32, tag="s", name="s_t")
                                nc.vector.tensor_add(
                                    s_t,
                                    asf(ins[("v", c)][:, sl]),
                                    asf(ins[("i", c)][:, sl]),
                                )
                                nc.vector.tensor_add(outs[c][:, sl], ps2, s_t)
                    for c in range(CCH):
                        nc.sync.dma_start(
                            out[b, c * 128 : (c + 1) * 128, jsl], outs[c]
                        )
    nc.compile()
    return nc


def _build_v7(mm_dt: str) -> bass.Bass:
    """bf16 streaming rewrite.

    Structure (per core: 2 images, split into pixel ranges of NR):
      - One DMA push per (tensor, range): dram [256, NR] -> SBUF [128, 2, NR]
        via a (g p) x -> p g x access pattern. 16 input pushes + 8 output
        pushes total (fixed ~0.6us issue cost per push makes fewer/bigger
        pushes strictly better).
      - Every tile is uniquely allocated (SBUF is big enough at bf16 to hold
        ALL inputs + outputs) -> zero WAR hazards, so every input push is
        issued up front and the input stream runs at full DMA rate.
      - Per range: DVE bypass add (vis+ir, optionally split with gpsimd),
        ko-outer mm1 (4 LDWEIGHTS instead of 8), ACT relu+b1, mm2 with b2 via
        ones-row, DVE psum adds, one output push on the ACT queue.
      - Optional PE keep-warm dummy matmuls per range: the PE p-state drops
        to 1.2 GHz after any idle gap and needs 3us of continuous work to
        re-reach 2.4 GHz; dummies bridge the gaps left by the DMA cadence.
    """
    d = BF16
    NR = int(os.environ.get("KNR", "1024"))
    NCH = 512
    nch = NR // NCH
    n_warm = int(os.environ.get("KWARM", "12"))
    n_dummy = int(os.environ.get("KDUMMY", "0"))
    gp_bypass = int(os.environ.get("KGP", "0"))
    ps1_bufs = int(os.environ.get("KPS1", "3"))
    ps2_bufs = int(os.environ.get("KPS2", "2"))

    nc = bacc.Bacc(
        "TRN2", target_bir_lowering=False, debug=False, num_devices=N_CORES
    )
    vis = nc.dram_tensor("vis", [B_PER_CORE, C, HWPIX], d, kind="ExternalInput")
    ir = nc.dram_tensor("ir", [B_PER_CORE, C, HWPIX], d, kind="ExternalInput")
    w1t = nc.dram_tensor("w1t", [KO, 128, HID1], d, kind="ExternalInput")
    b1 = nc.dram_tensor("b1", [HID1, 1], F32, kind="ExternalInput")
    w2t = nc.dram_tensor("w2t", [HID1, C], d, kind="ExternalInput")
    iden = nc.dram_tensor("iden", [128, 4 * 128], d, kind="ExternalInput")
    out = nc.dram_tensor("out", [B_PER_CORE, C, HWPIX], d, kind="ExternalOutput")

    ranges = [(b, j) for b in range(B_PER_CORE) for j in range(HWPIX // NR)]

    with TileContext(nc) as tc:
        with (
            tc.tile_pool(name="consts", bufs=1) as cpool,
            tc.tile_pool(name="data", bufs=1) as dpool,
            tc.tile_pool(name="ps1", bufs=ps1_bufs, space="PSUM") as ps1pool,
            tc.tile_pool(name="ps2", bufs=ps2_bufs, space="PSUM") as ps2pool,
            tc.tile_pool(name="psw", bufs=1, space="PSUM") as pswpool,
        ):
            # consts ride the ACT queue so the sync queue starts with real data
            iden_sb = cpool.tile([128, 4 * 128], d, tag="iden")
            nc.scalar.dma_start(iden_sb, iden[:, :])
            w1t_sb = cpool.tile([128, KO, HID1], d, tag="w1t")
            nc.scalar.dma_start(w1t_sb, w1t[:, :, :].rearrange("ko p m -> p ko m"))
            w2t_sb = cpool.tile([HID1, C], d, tag="w2t")
            nc.scalar.dma_start(w2t_sb, w2t[:, :])
            b1_sb = cpool.tile([HID1, 1], F32, tag="b1")
            nc.scalar.dma_start(b1_sb, b1[:, :])

            # all input pushes up front; unique buffers -> no WAR waits
            in_v, in_i = {}, {}
            for r, (b, j) in enumerate(ranges):
                jsl = slice(j * NR, (j + 1) * NR)
                tv = dpool.tile([128, 2, NR], d, tag=f"v{r}", name=f"v{r}")
                ti = dpool.tile([128, 2, NR], d, tag=f"i{r}", name=f"i{r}")
                nc.sync.dma_start(
                    tv, vis[b, :, jsl].rearrange("(g p) x -> p g x", g=2)
                )
                nc.sync.dma_start(
                    ti, ir[b, :, jsl].rearrange("(g p) x -> p g x", g=2)
                )
                in_v[r], in_i[r] = tv, ti

            warm_ps = pswpool.tile([128, NCH], F32, tag="warm", name="warm_ps")
            for _ in range(n_warm):
                nc.tensor.matmul(
                    warm_ps,
                    lhsT=iden_sb[:, :128],
                    rhs=iden_sb[:, :NCH],
                    start=True,
                    stop=True,
                )

            for r, (b, j) in enumerate(ranges):
                jsl = slice(j * NR, (j + 1) * NR)
                tv, ti = in_v[r], in_i[r]
                outt = dpool.tile([128, 2, NR], d, tag=f"o{r}", name=f"o{r}")
                if gp_bypass:
                    nc.vector.tensor_add(outt[:, 0], tv[:, 0], ti[:, 0])
                    nc.gpsimd.tensor_add(outt[:, 1], tv[:, 1], ti[:, 1])
                else:
                    nc.vector.tensor_add(outt, tv, ti)
                x_t = dpool.tile([HID1, NR], d, tag=f"x{r}", name=f"x{r}")
                ps1s = [
                    ps1pool.tile([HID1, NCH], F32, tag="ps1", name="ps1")
                    for _ in range(nch)
                ]
                for ko in range(KO):
                    src = tv if ko < 2 else ti
                    g = ko % 2
                    for ch in range(nch):
                        nc.tensor.matmul(
                            ps1s[ch],
                            lhsT=w1t_sb[:, ko],
                            rhs=src[:, g, ch * NCH : (ch + 1) * NCH],
                            start=(ko == 0),
                            stop=(ko == KO - 1),
                        )
                for ch in range(nch):
                    nc.scalar.activation(
                        x_t[:, ch * NCH : (ch + 1) * NCH],
                        ps1s[ch],
                        mybir.ActivationFunctionType.Relu,
                        bias=b1_sb[:, 0:1],
                    )
                for c in range(CCH):
                    ps2 = ps2pool.tile([128, NR], F32, tag="ps2", name="ps2")
                    for q in range(nch):
                        nc.tensor.matmul(
                            ps2[:, q * NCH : (q + 1) * NCH],
                            lhsT=w2t_sb[:, c * 128 : (c + 1) * 128],
                            rhs=x_t[:, q * NCH : (q + 1) * NCH],
                            start=True,
                            stop=True,
                        )
                    nc.vector.tensor_add(outt[:, c], outt[:, c], ps2)
                for _ in range(n_dummy):
                    nc.tensor.matmul(
                        warm_ps,
                        lhsT=iden_sb[:, :128],
                        rhs=iden_sb[:, :NCH],
                        start=True,
                        stop=True,
                    )
                nc.scalar.dma_start(
                    out[b, :, jsl].rearrange("(g p) x -> p g x", g=2), outt
                )
    nc.compile()
    return nc


def _get_nc(mm_dt: str, variant: str) -> bass.Bass:
    key = ("nc", mm_dt, variant)
    if key not in _cache:
        if variant == "v7":
            _cache[key] = _build_v7(mm_dt)
        else:
            _cache[key] = _build(mm_dt, variant)
    return _cache[key]


def kernel(
    visible_features: np.ndarray,
    infrared_features: np.ndarray,
    W1: np.ndarray,
    b1: np.ndarray,
    W2: np.ndarray,
    b2: np.ndarray,
    _mm_dt: str = "bf16",
    _variant: str = "v5",
    _trace: bool = False,
) -> np.ndarray:
    nc = _get_nc(_mm_dt, _variant)

    if _mm_dt == "bf16":
        import ml_dtypes

        io_np = ml_dtypes.bfloat16
    else:
        io_np = np.float32

    vis = np.ascontiguousarray(
        np.asarray(visible_features).astype(io_np).reshape(B, C, HWPIX)
    )
    ir = np.ascontiguousarray(
        np.asarray(infrared_features).astype(io_np).reshape(B, C, HWPIX)
    )

    w1t = np.zeros((2 * C, HID1), dtype=np.float32)
    w1t[:, :HID] = W1.astype(np.float32).T
    w1t = np.ascontiguousarray(w1t.reshape(KO, 128, HID1)).astype(io_np)
    b1r = np.ones((HID1, 1), dtype=np.float32)
    b1r[:HID, 0] = b1.astype(np.float32)
    w2t = np.zeros((HID1, C), dtype=np.float32)
    w2t[:HID] = W2.astype(np.float32).T
    w2t[HID] = b2.astype(np.float32)
    w2t = w2t.astype(io_np)
    iden = np.zeros((128, 4 * 128), dtype=np.float32)
    iden[:, :128] = np.eye(128, dtype=np.float32)
    iden = iden.astype(io_np)

    in_maps = []
    for core in range(N_CORES):
        bsl = slice(core * B_PER_CORE, (core + 1) * B_PER_CORE)
        in_maps.append(
            {
                "vis": vis[bsl],
                "ir": ir[bsl],
                "w1t": w1t,
                "b1": b1r,
                "w2t": w2t,
                "iden": iden,
            }
        )

    res = bass_utils.run_bass_kernel_spmd(
        nc, in_maps, core_ids=list(range(N_CORES)), trace=_trace
    )
    if _trace:
        kernel.last_results = res
    outs = [np.asarray(r["out"]).astype(np.float32) for r in res.results]
    return np.concatenate(outs, axis=0).reshape(B, C, H, W)



# revision 14
# speedup vs baseline: 1.5797x; 1.0200x over previous
"""CrossModalFeatureInteraction kernel for Trainium2 (Bass/Tile), 8 NeuronCores.

Computation (per pixel, per batch):
    combined = concat([vis, ir], channel)              # [512]
    x        = relu(W1 @ combined + b1)                # [32]
    residual = W2 @ x + b2                             # [256]
    out      = vis + ir + residual                     # [256]

Sharding: data-parallel over batch. B=16 -> 2 images per core on 8 cores.
Weights are tiny and replicated. Each core streams its 2 images through
SBUF in pixel supertiles; 1x1 convs are matmuls with channels as the
contraction dim and pixels as the moving free dim.

Engine budget tricks (target regime is memory; DMA ~77us/core is the
roofline, so every other engine must stay well under it):
  - Matmuls run in float32r: full-rate (1 col/cycle) PE mode on fp32 bits.
  - b1 rides as the activation bias; an all-zero 33rd W1 column plus
    bias=1.0 makes x's 33rd row == 1.0, so b2 rides as the 33rd row of
    W2 (K=33 second matmul). No separate bias pass.
  - Each supertile runs in two phases: all first-layer matmuls + relus
    into one batched x tile first, then all second-layer matmuls. The
    PE never waits on the ACT relu this way (its consumer runs ~4 tile
    slots behind the producer), so it stays dense and HAM-warm.
  - variant "hybrid": the ir half of the bypass is added by the PE (an
    identity matmul accumulating into the residual PSUM bank), leaving
    DVE one add per output tile. variant "dve": both bypass adds on DVE
    (fp32 tensor_tensor is 1x mode, so this doubles DVE time but frees
    the PE).
"""

import numpy as np

import concourse.bass as bass
import concourse.mybir as mybir
from concourse import bacc, bass_utils
from concourse.tile import TileContext

# Problem shape (hardcoded per contract)
B, C, H, W = 16, 256, 64, 64
HID = 32
HWPIX = H * W          # 4096 pixels per image
N_CORES = 8
B_PER_CORE = B // N_CORES  # 2

import os
NBIG = int(os.environ.get("KNBIG", "2048"))  # pixels per DMA supertile (1 MiB tiles)
INBUFS = int(os.environ.get("KINBUFS", "4"))  # input-tile double-buffer depth
NT = 512               # matmul moving free dim (one PSUM bank of fp32)
KO = 4                 # 512 combined channels / 128 partitions
CCH = 2                # 256 output channels / 128 partitions
HID1 = HID + 1         # hidden + ones row (carries b2 through matmul 2)

F32 = mybir.dt.float32
F32R = mybir.dt.float32r
BF16 = mybir.dt.bfloat16

_cache = {}


def _build(mm_dt: str, variant: str) -> bass.Bass:
    if mm_dt == "bf16":
        d = BF16
    elif mm_dt == "f32r":
        d = F32R
    else:
        d = F32
    bf16 = d is BF16
    # IO dtype for the output tensor / out tiles: bf16 halves store traffic
    # (rel-err budget is 2e-2; all-bf16 IO lands at ~2.4e-3).
    od = BF16 if bf16 else F32
    hybrid = variant == "hybrid"

    nc = bacc.Bacc(
        "TRN2", target_bir_lowering=False, debug=False, num_devices=N_CORES
    )
    vis = nc.dram_tensor("vis", [B_PER_CORE, C, HWPIX], d, kind="ExternalInput")
    ir = nc.dram_tensor("ir", [B_PER_CORE, C, HWPIX], d, kind="ExternalInput")
    w1t = nc.dram_tensor("w1t", [KO, 128, HID1], d, kind="ExternalInput")
    b1 = nc.dram_tensor("b1", [HID1, 1], F32, kind="ExternalInput")
    w2t = nc.dram_tensor("w2t", [HID1, C], d, kind="ExternalInput")
    iden = nc.dram_tensor("iden", [128, 4 * 128], d, kind="ExternalInput")
    out = nc.dram_tensor("out", [B_PER_CORE, C, HWPIX], od, kind="ExternalOutput")

    # DVE operand view: f32r tiles must be bitcast to f32; bf16 tiles are
    # consumed natively (16-bit DVE ops run at 2x).
    asf = (lambda t: t) if bf16 else (lambda t: t.bitcast(F32))

    big = variant in ("big", "v4", "v5", "v5t")
    ko_outer = variant == "v4"
    halves = variant in ("v5", "v5t", "v6")
    v6 = variant == "v6"
    tweak_ends = variant == "v5t"
    last_bj = (B_PER_CORE - 1, HWPIX // NBIG - 1)
    with TileContext(nc) as tc:
        with (
            tc.tile_pool(name="consts", bufs=1) as cpool,
            tc.tile_pool(name="inbuf", bufs=INBUFS) as inpool,
            tc.tile_pool(name="work", bufs=2) as wpool,
            tc.tile_pool(name="outbuf", bufs=2) as opool,
            tc.tile_pool(
                name="ps1",
                bufs=int(os.environ.get("KPS1", "4" if big else "3")),
                space="PSUM",
            ) as ps1pool,
            tc.tile_pool(
                name="ps2", bufs=int(os.environ.get("KPS2", "2")), space="PSUM"
            ) as ps2pool,
        ):
            # consts ride the ACT HWDGE queue in v5 so the SP queue's very
            # first entries are real input data
            cdma = nc.scalar if halves else nc.sync
            # iden first: it feeds the PE warm-up, so it must land as early
            # as the engine preamble allows
            iden_sb = cpool.tile([128, 4 * 128], d, tag="iden")
            cdma.dma_start(iden_sb, iden[:, :])
            w1t_sb = cpool.tile([128, KO, HID1], d, tag="w1t")
            cdma.dma_start(w1t_sb, w1t[:, :, :].rearrange("ko p m -> p ko m"))
            w2t_sb = cpool.tile([HID1, C], d, tag="w2t")
            cdma.dma_start(w2t_sb, w2t[:, :])
            b1_sb = cpool.tile([HID1, 1], F32, tag="b1")
            cdma.dma_start(b1_sb, b1[:, :])

            # PE warm-up: HAM throttles the PE to 1.2 GHz until it sees
            # ~3.4us of sustained matmul activity. Run dummy matmuls on
            # const data during the startup DMA window so the real work
            # starts (and stays) at 2.4 GHz. All into one PSUM tile
            # (same-bank WAW keeps them back-to-back on the PE, and a
            # single allocation avoids pool-rotation stalls). Results are
            # never read.
            n_warm = int(os.environ.get("KWARM", "12"))
            if n_warm:
                warm_ps = ps2pool.tile([128, 2 * NT], F32, tag="ps2", name="warm_ps")
                for w in range(n_warm):
                    nc.tensor.matmul(
                        warm_ps[:, :NT],
                        lhsT=iden_sb[:, :128],
                        rhs=iden_sb[:, :NT],
                        start=True,
                        stop=True,
                    )

            def supertile_v6(b, j):
                """Per-half software pipeline: DMA(h) -> bypass-add(h) +
                first-layer(h) -> second-layer(h) -> out-DMA(h). The tail
                of the whole kernel is just one half-chunk's phase B."""
                ins = {}
                for nm in ("v", "i"):
                    for c in range(CCH):
                        ins[(nm, c)] = inpool.tile(
                            [128, NBIG], d, tag=f"in_{nm}{c}", name=f"in_{nm}{c}"
                        )
                outs = [
                    opool.tile([128, NBIG], od, tag=f"out{c}", name=f"outt{c}")
                    for c in range(CCH)
                ]
                x_big = wpool.tile([HID1, NBIG], d, tag="x", name="x_big")
                rhs_order = [("v", 0), ("v", 1), ("i", 0), ("i", 1)]
                for h in range(2):
                    hw_ = NBIG // 2
                    hs = slice(h * hw_, (h + 1) * hw_)
                    gs = slice(j * NBIG + h * hw_, j * NBIG + (h + 1) * hw_)
                    for nm, dram in (("v", vis), ("i", ir)):
                        for c in range(CCH):
                            nc.sync.dma_start(
                                ins[(nm, c)][:, hs],
                                dram[b, c * 128 : (c + 1) * 128, gs],
                            )
                    for c in range(CCH):
                        nc.vector.tensor_add(
                            outs[c][:, hs],
                            asf(ins[("v", c)][:, hs]),
                            asf(ins[("i", c)][:, hs]),
                        )
                    for js in (2 * h, 2 * h + 1):
                        sl = slice(js * NT, (js + 1) * NT)
                        ps1 = ps1pool.tile([HID1, NT], F32, tag="ps1", name="ps1")
                        for ko, key in enumerate(rhs_order):
                            nc.tensor.matmul(
                                ps1,
                                lhsT=w1t_sb[:, ko],
                                rhs=ins[key][:, sl],
                                start=(ko == 0),
                                stop=(ko == KO - 1),
                            )
                        nc.scalar.activation(
                            x_big[:, sl], ps1,
                            mybir.ActivationFunctionType.Relu,
                            bias=b1_sb[:, 0:1],
                        )
                    for c in range(CCH):
                        ps2 = ps2pool.tile([128, 2 * NT], F32, tag="ps2", name="ps2")
                        for q in range(2):
                            xsl = slice((2 * h + q) * NT, (2 * h + q + 1) * NT)
                            nc.tensor.matmul(
                                ps2[:, q * NT : (q + 1) * NT],
                                lhsT=w2t_sb[:, c * 128 : (c + 1) * 128],
                                rhs=x_big[:, xsl],
                                start=True,
                                stop=True,
                            )
                        nc.vector.tensor_add(outs[c][:, hs], outs[c][:, hs], ps2)
                        nc.scalar.dma_start(
                            out[b, c * 128 : (c + 1) * 128, gs], outs[c][:, hs]
                        )

            for b in range(B_PER_CORE):
                for j in range(HWPIX // NBIG):
                    if v6:
                        supertile_v6(b, j)
                        continue
                    jsl = slice(j * NBIG, (j + 1) * NBIG)
                    ins = {}
                    if halves:
                        # interleaved half-tile DMAs: the single SP queue is
                        # FIFO, so this ordering lands a complete half-set
                        # (all 4 inputs' columns h) every ~5.6us instead of
                        # one whole input every ~2.8us. PE can start each
                        # js-pair as its half-set arrives; idle slivers stay
                        # under the ~3.4us HAM window.
                        for nm, dram in (("v", vis), ("i", ir)):
                            for c in range(CCH):
                                ins[(nm, c)] = inpool.tile(
                                    [128, NBIG], d,
                                    tag=f"in_{nm}{c}", name=f"in_{nm}{c}",
                                )
                        # the very first supertile splits its input halves
                        # across both HWDGE queues (outputs don't need the
                        # ACT queue yet) so the first half-set lands in
                        # ~2.8us instead of ~5.6us
                        dual = tweak_ends and b == 0 and j == 0
                        for h in range(2):
                            hs = slice(h * (NBIG // 2), (h + 1) * (NBIG // 2))
                            gs = slice(
                                j * NBIG + h * (NBIG // 2),
                                j * NBIG + (h + 1) * (NBIG // 2),
                            )
                            for nm, dram in (("v", vis), ("i", ir)):
                                eng = nc.scalar if (dual and nm == "i") else nc.sync
                                for c in range(CCH):
                                    eng.dma_start(
                                        ins[(nm, c)][:, hs],
                                        dram[b, c * 128 : (c + 1) * 128, gs],
                                    )
                    else:
                        for nm, dram in (("v", vis), ("i", ir)):
                            for c in range(CCH):
                                t = inpool.tile(
                                    [128, NBIG], d,
                                    tag=f"in_{nm}{c}", name=f"in_{nm}{c}",
                                )
                                nc.sync.dma_start(
                                    t, dram[b, c * 128 : (c + 1) * 128, jsl]
                                )
                                ins[(nm, c)] = t
                    outs = [
                        opool.tile([128, NBIG], od, tag=f"out{c}", name=f"outt{c}")
                        for c in range(CCH)
                    ]
                    if big:
                        # bypass sum first: outs[c] = vis_c + ir_c in big DVE
                        # ops (fewer instructions, runs while PE does phase A)
                        for c in range(CCH):
                            if halves:
                                for h in range(2):
                                    hs = slice(
                                        h * (NBIG // 2), (h + 1) * (NBIG // 2)
                                    )
                                    nc.vector.tensor_add(
                                        outs[c][:, hs],
                                        asf(ins[("v", c)][:, hs]),
                                        asf(ins[("i", c)][:, hs]),
                                    )
                            else:
                                nc.vector.tensor_add(
                                    outs[c],
                                    asf(ins[("v", c)]),
                                    asf(ins[("i", c)]),
                                )
                    # Phase A: first layer for the whole supertile.
                    # x rows 0..31 = relu(W1@c + b1); row 32 = relu(0+1) = 1
                    x_big = wpool.tile([HID1, NBIG], d, tag="x", name="x_big")
                    rhs_order = [("v", 0), ("v", 1), ("i", 0), ("i", 1)]
                    if ko_outer:
                        # K-outer: the single HWDGE queue delivers the four
                        # input tiles sequentially (~2.8us apart), so consume
                        # each K chunk as it lands instead of waiting for all
                        # four. Keeps PE idle slivers under the ~3.4us HAM
                        # window -> matmuls stay at 2.4 GHz.
                        ps1s = [
                            ps1pool.tile([HID1, NT], F32, tag="ps1", name="ps1")
                            for _ in range(NBIG // NT)
                        ]
                        for ko, key in enumerate(rhs_order):
                            for js in range(NBIG // NT):
                                sl = slice(js * NT, (js + 1) * NT)
                                nc.tensor.matmul(
                                    ps1s[js],
                                    lhsT=w1t_sb[:, ko],
                                    rhs=ins[key][:, sl],
                                    start=(ko == 0),
                                    stop=(ko == KO - 1),
                                )
                        for js in range(NBIG // NT):
                            sl = slice(js * NT, (js + 1) * NT)
                            nc.scalar.activation(
                                x_big[:, sl], ps1s[js],
                                mybir.ActivationFunctionType.Relu,
                                bias=b1_sb[:, 0:1],
                            )
                    else:
                        for js in range(NBIG // NT):
                            sl = slice(js * NT, (js + 1) * NT)
                            ps1 = ps1pool.tile([HID1, NT], F32, tag="ps1", name="ps1")
                            for ko, key in enumerate(rhs_order):
                                nc.tensor.matmul(
                                    ps1,
                                    lhsT=w1t_sb[:, ko],
                                    rhs=ins[key][:, sl],
                                    start=(ko == 0),
                                    stop=(ko == KO - 1),
                                )
                            nc.scalar.activation(
                                x_big[:, sl], ps1, mybir.ActivationFunctionType.Relu,
                                bias=b1_sb[:, 0:1],
                            )
                    # Phase B: second layer + bypass adds.
                    if tweak_ends and (b, j) == last_bj:
                        # final supertile: NT-granular phase B so the kernel
                        # tail is one small chunk's matmul+add+DMA chain
                        for js in range(NBIG // NT):
                            sl = slice(js * NT, (js + 1) * NT)
                            for c in range(CCH):
                                ps2 = ps2pool.tile(
                                    [128, 2 * NT], F32, tag="ps2", name="ps2"
                                )
                                nc.tensor.matmul(
                                    ps2[:, :NT],
                                    lhsT=w2t_sb[:, c * 128 : (c + 1) * 128],
                                    rhs=x_big[:, sl],
                                    start=True,
                                    stop=True,
                                )
                                nc.vector.tensor_add(
                                    outs[c][:, sl], outs[c][:, sl], ps2[:, :NT]
                                )
                                nc.scalar.dma_start(
                                    out[
                                        b,
                                        c * 128 : (c + 1) * 128,
                                        j * NBIG + js * NT : j * NBIG
                                        + (js + 1) * NT,
                                    ],
                                    outs[c][:, sl],
                                )
                        continue
                    if big:
                        # mm2 in [128, 2*NT] psum chunks; one in-place DVE
                        # add per chunk (outs += residual+b2), then DMA the
                        # finished half straight out.
                        for h in range(NBIG // (2 * NT)):
                            hsl = slice(h * 2 * NT, (h + 1) * 2 * NT)
                            for c in range(CCH):
                                ps2 = ps2pool.tile(
                                    [128, 2 * NT], F32, tag="ps2", name="ps2"
                                )
                                for q in range(2):
                                    xsl = slice(
                                        (2 * h + q) * NT, (2 * h + q + 1) * NT
                                    )
                                    nc.tensor.matmul(
                                        ps2[:, q * NT : (q + 1) * NT],
                                        lhsT=w2t_sb[:, c * 128 : (c + 1) * 128],
                                        rhs=x_big[:, xsl],
                                        start=True,
                                        stop=True,
                                    )
                                nc.vector.tensor_add(
                                    outs[c][:, hsl], outs[c][:, hsl], ps2
                                )
                                # out-DMAs ride the ACT HWDGE queue so they
                                # never delay input prefetch on the SP queue
                                nc.scalar.dma_start(
                                    out[
                                        b,
                                        c * 128 : (c + 1) * 128,
                                        j * NBIG + h * 2 * NT : j * NBIG
                                        + (h + 1) * 2 * NT,
                                    ],
                                    outs[c][:, hsl],
                                )
                        continue
                    for js in range(NBIG // NT):
                        sl = slice(js * NT, (js + 1) * NT)
                        for c in range(CCH):
                            ps2 = ps2pool.tile(
                                [128, NT], F32, tag=f"ps2_{c}", name=f"ps2_{c}"
                            )
                            if hybrid:
                                # psum2 = I.T @ ir_c  (ir bypass on the PE)
                                nc.tensor.matmul(
                                    ps2,
                                    lhsT=iden_sb[:, :128],
                                    rhs=ins[("i", c)][:, sl],
                                    start=True,
                                    stop=False,
                                )
                            # psum2 += [W2; b2].T @ [x; 1] = residual + b2
                            nc.tensor.matmul(
                                ps2,
                                lhsT=w2t_sb[:, c * 128 : (c + 1) * 128],
                                rhs=x_big[:, sl],
                                start=not hybrid,
                                stop=True,
                            )
                            if hybrid:
                                nc.vector.tensor_add(
                                    outs[c][:, sl],
                                    ps2,
                                    asf(ins[("v", c)][:, sl]),
                                )
                            else:
                                s_t = wpool.tile([128, NT], F32, tag="s", name="s_t")
                                nc.vector.tensor_add(
                                    s_t,
                                    asf(ins[("v", c)][:, sl]),
                                    asf(ins[("i", c)][:, sl]),
                                )
                                nc.vector.tensor_add(outs[c][:, sl], ps2, s_t)
                    for c in range(CCH):
                        nc.sync.dma_start(
                            out[b, c * 128 : (c + 1) * 128, jsl], outs[c]
                        )
    nc.compile()
    return nc


def _build_v7(mm_dt: str) -> bass.Bass:
    """bf16 streaming rewrite.

    Structure (per core: 2 images, split into pixel ranges of NR):
      - One DMA push per (tensor, range): dram [256, NR] -> SBUF [128, 2, NR]
        via a (g p) x -> p g x access pattern. 16 input pushes + 8 output
        pushes total (fixed ~0.6us issue cost per push makes fewer/bigger
        pushes strictly better).
      - Every tile is uniquely allocated (SBUF is big enough at bf16 to hold
        ALL inputs + outputs) -> zero WAR hazards, so every input push is
        issued up front and the input stream runs at full DMA rate.
      - Per range: DVE bypass add (vis+ir, optionally split with gpsimd),
        ko-outer mm1 (4 LDWEIGHTS instead of 8), ACT relu+b1, mm2 with b2 via
        ones-row, DVE psum adds, one output push on the ACT queue.
      - Optional PE keep-warm dummy matmuls per range: the PE p-state drops
        to 1.2 GHz after any idle gap and needs 3us of continuous work to
        re-reach 2.4 GHz; dummies bridge the gaps left by the DMA cadence.
    """
    d = BF16
    NR = int(os.environ.get("KNR", "1024"))
    NCH = 512
    nch = NR // NCH
    n_warm = int(os.environ.get("KWARM", "12"))
    n_dummy = int(os.environ.get("KDUMMY", "0"))
    gp_bypass = int(os.environ.get("KGP", "0"))
    ps1_bufs = int(os.environ.get("KPS1", "3"))
    ps2_bufs = int(os.environ.get("KPS2", "2"))

    nc = bacc.Bacc(
        "TRN2", target_bir_lowering=False, debug=False, num_devices=N_CORES
    )
    vis = nc.dram_tensor("vis", [B_PER_CORE, C, HWPIX], d, kind="ExternalInput")
    ir = nc.dram_tensor("ir", [B_PER_CORE, C, HWPIX], d, kind="ExternalInput")
    w1t = nc.dram_tensor("w1t", [KO, 128, HID1], d, kind="ExternalInput")
    b1 = nc.dram_tensor("b1", [HID1, 1], F32, kind="ExternalInput")
    w2t = nc.dram_tensor("w2t", [HID1, C], d, kind="ExternalInput")
    iden = nc.dram_tensor("iden", [128, 4 * 128], d, kind="ExternalInput")
    out = nc.dram_tensor("out", [B_PER_CORE, C, HWPIX], d, kind="ExternalOutput")

    ranges = [(b, j) for b in range(B_PER_CORE) for j in range(HWPIX // NR)]

    from contextlib import ExitStack

    with TileContext(nc) as tc, ExitStack() as es:
        if True:
            cpool = es.enter_context(tc.tile_pool(name="consts", bufs=1))
            dpool = es.enter_context(tc.tile_pool(name="data", bufs=1))
            ps1pool = es.enter_context(
                tc.tile_pool(name="ps1", bufs=ps1_bufs, space="PSUM")
            )
            ps2pool = es.enter_context(
                tc.tile_pool(name="ps2", bufs=ps2_bufs, space="PSUM")
            )
            pswpool = (
                es.enter_context(tc.tile_pool(name="psw", bufs=1, space="PSUM"))
                if (n_warm or n_dummy)
                else None
            )
            # consts ride the ACT queue so the sync queue starts with real data
            iden_sb = cpool.tile([128, 4 * 128], d, tag="iden")
            nc.scalar.dma_start(iden_sb, iden[:, :])
            w1t_sb = cpool.tile([128, KO, HID1], d, tag="w1t")
            nc.scalar.dma_start(w1t_sb, w1t[:, :, :].rearrange("ko p m -> p ko m"))
            w2t_sb = cpool.tile([HID1, C], d, tag="w2t")
            nc.scalar.dma_start(w2t_sb, w2t[:, :])
            b1_sb = cpool.tile([HID1, 1], F32, tag="b1")
            nc.scalar.dma_start(b1_sb, b1[:, :])

            # all input pushes up front; unique buffers -> no WAR waits
            in_v, in_i = {}, {}
            for r, (b, j) in enumerate(ranges):
                jsl = slice(j * NR, (j + 1) * NR)
                tv = dpool.tile([128, 2, NR], d, tag=f"v{r}", name=f"v{r}")
                ti = dpool.tile([128, 2, NR], d, tag=f"i{r}", name=f"i{r}")
                nc.sync.dma_start(
                    tv, vis[b, :, jsl].rearrange("(g p) x -> p g x", g=2)
                )
                nc.sync.dma_start(
                    ti, ir[b, :, jsl].rearrange("(g p) x -> p g x", g=2)
                )
                in_v[r], in_i[r] = tv, ti

            warm_ps = None
            if pswpool is not None:
                warm_ps = pswpool.tile([128, NCH], F32, tag="warm", name="warm_ps")
            for _ in range(n_warm):
                nc.tensor.matmul(
                    warm_ps,
                    lhsT=iden_sb[:, :128],
                    rhs=iden_sb[:, :NCH],
                    start=True,
                    stop=True,
                )

            for r, (b, j) in enumerate(ranges):
                jsl = slice(j * NR, (j + 1) * NR)
                tv, ti = in_v[r], in_i[r]
                outt = dpool.tile([128, 2, NR], d, tag=f"o{r}", name=f"o{r}")
                if gp_bypass:
                    nc.vector.tensor_add(outt[:, 0], tv[:, 0], ti[:, 0])
                    nc.gpsimd.tensor_add(outt[:, 1], tv[:, 1], ti[:, 1])
                else:
                    nc.vector.tensor_add(outt, tv, ti)
                x_t = dpool.tile([HID1, NR], d, tag=f"x{r}", name=f"x{r}")
                ps1s = [
                    ps1pool.tile([HID1, NCH], F32, tag="ps1", name="ps1")
                    for _ in range(nch)
                ]
                for ko in range(KO):
                    src = tv if ko < 2 else ti
                    g = ko % 2
                    for ch in range(nch):
                        nc.tensor.matmul(
                            ps1s[ch],
                            lhsT=w1t_sb[:, ko],
                            rhs=src[:, g, ch * NCH : (ch + 1) * NCH],
                            start=(ko == 0),
                            stop=(ko == KO - 1),
                        )
                for ch in range(nch):
                    nc.scalar.activation(
                        x_t[:, ch * NCH : (ch + 1) * NCH],
                        ps1s[ch],
                        mybir.ActivationFunctionType.Relu,
                        bias=b1_sb[:, 0:1],
                    )
                for c in range(CCH):
                    ps2 = ps2pool.tile([128, NR], F32, tag="ps2", name="ps2")
                    for q in range(nch):
                        nc.tensor.matmul(
                            ps2[:, q * NCH : (q + 1) * NCH],
                            lhsT=w2t_sb[:, c * 128 : (c + 1) * 128],
                            rhs=x_t[:, q * NCH : (q + 1) * NCH],
                            start=True,
                            stop=True,
                        )
                    nc.vector.tensor_add(outt[:, c], outt[:, c], ps2)
                for _ in range(n_dummy):
                    nc.tensor.matmul(
                        warm_ps,
                        lhsT=iden_sb[:, :128],
                        rhs=iden_sb[:, :NCH],
                        start=True,
                        stop=True,
                    )
                nc.scalar.dma_start(
                    out[b, :, jsl].rearrange("(g p) x -> p g x", g=2), outt
                )
    nc.compile()
    return nc


def _get_nc(mm_dt: str, variant: str) -> bass.Bass:
    key = ("nc", mm_dt, variant)
    if key not in _cache:
        if variant == "v7":
            _cache[key] = _build_v7(mm_dt)
        else:
            _cache[key] = _build(mm_dt, variant)
    return _cache[key]


def kernel(
    visible_features: np.ndarray,
    infrared_features: np.ndarray,
    W1: np.ndarray,
    b1: np.ndarray,
    W2: np.ndarray,
    b2: np.ndarray,
    _mm_dt: str = "bf16",
    _variant: str = "v5",
    _trace: bool = False,
) -> np.ndarray:
    nc = _get_nc(_mm_dt, _variant)

    if _mm_dt == "bf16":
        import ml_dtypes

        io_np = ml_dtypes.bfloat16
    else:
        io_np = np.float32

    vis = np.ascontiguousarray(
        np.asarray(visible_features).astype(io_np).reshape(B, C, HWPIX)
    )
    ir = np.ascontiguousarray(
        np.asarray(infrared_features).astype(io_np).reshape(B, C, HWPIX)
    )

    w1t = np.zeros((2 * C, HID1), dtype=np.float32)
    w1t[:, :HID] = W1.astype(np.float32).T
    w1t = np.ascontiguousarray(w1t.reshape(KO, 128, HID1)).astype(io_np)
    b1r = np.ones((HID1, 1), dtype=np.float32)
    b1r[:HID, 0] = b1.astype(np.float32)
    w2t = np.zeros((HID1, C), dtype=np.float32)
    w2t[:HID] = W2.astype(np.float32).T
    w2t[HID] = b2.astype(np.float32)
    w2t = w2t.astype(io_np)
    iden = np.zeros((128, 4 * 128), dtype=np.float32)
    iden[:, :128] = np.eye(128, dtype=np.float32)
    iden = iden.astype(io_np)

    in_maps = []
    for core in range(N_CORES):
        bsl = slice(core * B_PER_CORE, (core + 1) * B_PER_CORE)
        in_maps.append(
            {
                "vis": vis[bsl],
                "ir": ir[bsl],
                "w1t": w1t,
                "b1": b1r,
                "w2t": w2t,
                "iden": iden,
            }
        )

    res = bass_utils.run_bass_kernel_spmd(
        nc, in_maps, core_ids=list(range(N_CORES)), trace=_trace
    )
    if _trace:
        kernel.last_results = res
    outs = [np.asarray(r["out"]).astype(np.float32) for r in res.results]
    return np.concatenate(outs, axis=0).reshape(B, C, H, W)



# revision 21
# speedup vs baseline: 1.5971x; 1.0110x over previous
"""CrossModalFeatureInteraction kernel for Trainium2 (Bass/Tile), 8 NeuronCores.

Computation (per pixel, per batch):
    combined = concat([vis, ir], channel)              # [512]
    x        = relu(W1 @ combined + b1)                # [32]
    residual = W2 @ x + b2                             # [256]
    out      = vis + ir + residual                     # [256]

Sharding: data-parallel over batch. B=16 -> 2 images per core on 8 cores.
Weights are tiny and replicated. Each core streams its 2 images through
SBUF in pixel supertiles; 1x1 convs are matmuls with channels as the
contraction dim and pixels as the moving free dim.

Engine budget tricks (target regime is memory; DMA ~77us/core is the
roofline, so every other engine must stay well under it):
  - Matmuls run in float32r: full-rate (1 col/cycle) PE mode on fp32 bits.
  - b1 rides as the activation bias; an all-zero 33rd W1 column plus
    bias=1.0 makes x's 33rd row == 1.0, so b2 rides as the 33rd row of
    W2 (K=33 second matmul). No separate bias pass.
  - Each supertile runs in two phases: all first-layer matmuls + relus
    into one batched x tile first, then all second-layer matmuls. The
    PE never waits on the ACT relu this way (its consumer runs ~4 tile
    slots behind the producer), so it stays dense and HAM-warm.
  - variant "hybrid": the ir half of the bypass is added by the PE (an
    identity matmul accumulating into the residual PSUM bank), leaving
    DVE one add per output tile. variant "dve": both bypass adds on DVE
    (fp32 tensor_tensor is 1x mode, so this doubles DVE time but frees
    the PE).
"""

import numpy as np

import concourse.bass as bass
import concourse.mybir as mybir
from concourse import bacc, bass_utils
from concourse.tile import TileContext

# Problem shape (hardcoded per contract)
B, C, H, W = 16, 256, 64, 64
HID = 32
HWPIX = H * W          # 4096 pixels per image
N_CORES = 8
B_PER_CORE = B // N_CORES  # 2

import os
NBIG = int(os.environ.get("KNBIG", "2048"))  # pixels per DMA supertile (1 MiB tiles)
INBUFS = int(os.environ.get("KINBUFS", "4"))  # input-tile double-buffer depth
NT = 512               # matmul moving free dim (one PSUM bank of fp32)
KO = 4                 # 512 combined channels / 128 partitions
CCH = 2                # 256 output channels / 128 partitions
HID1 = HID + 1         # hidden + ones row (carries b2 through matmul 2)

F32 = mybir.dt.float32
F32R = mybir.dt.float32r
BF16 = mybir.dt.bfloat16

_cache = {}


def _build(mm_dt: str, variant: str) -> bass.Bass:
    if mm_dt == "bf16":
        d = BF16
    elif mm_dt == "f32r":
        d = F32R
    else:
        d = F32
    bf16 = d is BF16
    # IO dtype for the output tensor / out tiles: bf16 halves store traffic
    # (rel-err budget is 2e-2; all-bf16 IO lands at ~2.4e-3).
    od = BF16 if bf16 else F32
    hybrid = variant == "hybrid"

    nc = bacc.Bacc(
        "TRN2", target_bir_lowering=False, debug=False, num_devices=N_CORES
    )
    vis = nc.dram_tensor("vis", [B_PER_CORE, C, HWPIX], d, kind="ExternalInput")
    ir = nc.dram_tensor("ir", [B_PER_CORE, C, HWPIX], d, kind="ExternalInput")
    w1t = nc.dram_tensor("w1t", [KO, 128, HID1], d, kind="ExternalInput")
    b1 = nc.dram_tensor("b1", [HID1, 1], F32, kind="ExternalInput")
    w2t = nc.dram_tensor("w2t", [HID1, C], d, kind="ExternalInput")
    iden = nc.dram_tensor("iden", [128, 4 * 128], d, kind="ExternalInput")
    out = nc.dram_tensor("out", [B_PER_CORE, C, HWPIX], od, kind="ExternalOutput")

    # DVE operand view: f32r tiles must be bitcast to f32; bf16 tiles are
    # consumed natively (16-bit DVE ops run at 2x).
    asf = (lambda t: t) if bf16 else (lambda t: t.bitcast(F32))

    big = variant in ("big", "v4", "v5", "v5t")
    ko_outer = variant == "v4"
    halves = variant in ("v5", "v5t", "v6")
    v6 = variant == "v6"
    tweak_ends = variant == "v5t"
    last_bj = (B_PER_CORE - 1, HWPIX // NBIG - 1)
    with TileContext(nc) as tc:
        with (
            tc.tile_pool(name="consts", bufs=1) as cpool,
            tc.tile_pool(name="inbuf", bufs=INBUFS) as inpool,
            tc.tile_pool(name="work", bufs=2) as wpool,
            tc.tile_pool(name="outbuf", bufs=2) as opool,
            tc.tile_pool(
                name="ps1",
                bufs=int(os.environ.get("KPS1", "4" if big else "3")),
                space="PSUM",
            ) as ps1pool,
            tc.tile_pool(
                name="ps2", bufs=int(os.environ.get("KPS2", "2")), space="PSUM"
            ) as ps2pool,
        ):
            # consts ride the ACT HWDGE queue in v5 so the SP queue's very
            # first entries are real input data
            cdma = nc.scalar if halves else nc.sync
            # iden first: it feeds the PE warm-up, so it must land as early
            # as the engine preamble allows
            iden_sb = cpool.tile([128, 4 * 128], d, tag="iden")
            cdma.dma_start(iden_sb, iden[:, :])
            w1t_sb = cpool.tile([128, KO, HID1], d, tag="w1t")
            cdma.dma_start(w1t_sb, w1t[:, :, :].rearrange("ko p m -> p ko m"))
            w2t_sb = cpool.tile([HID1, C], d, tag="w2t")
            cdma.dma_start(w2t_sb, w2t[:, :])
            b1_sb = cpool.tile([HID1, 1], F32, tag="b1")
            cdma.dma_start(b1_sb, b1[:, :])

            # PE warm-up: HAM throttles the PE to 1.2 GHz until it sees
            # ~3.4us of sustained matmul activity. Run dummy matmuls on
            # const data during the startup DMA window so the real work
            # starts (and stays) at 2.4 GHz. All into one PSUM tile
            # (same-bank WAW keeps them back-to-back on the PE, and a
            # single allocation avoids pool-rotation stalls). Results are
            # never read.
            n_warm = int(os.environ.get("KWARM", "12"))
            if n_warm:
                warm_ps = ps2pool.tile([128, 2 * NT], F32, tag="ps2", name="warm_ps")
                for w in range(n_warm):
                    nc.tensor.matmul(
                        warm_ps[:, :NT],
                        lhsT=iden_sb[:, :128],
                        rhs=iden_sb[:, :NT],
                        start=True,
                        stop=True,
                    )

            def supertile_v6(b, j):
                """Per-half software pipeline: DMA(h) -> bypass-add(h) +
                first-layer(h) -> second-layer(h) -> out-DMA(h). The tail
                of the whole kernel is just one half-chunk's phase B."""
                ins = {}
                for nm in ("v", "i"):
                    for c in range(CCH):
                        ins[(nm, c)] = inpool.tile(
                            [128, NBIG], d, tag=f"in_{nm}{c}", name=f"in_{nm}{c}"
                        )
                outs = [
                    opool.tile([128, NBIG], od, tag=f"out{c}", name=f"outt{c}")
                    for c in range(CCH)
                ]
                x_big = wpool.tile([HID1, NBIG], d, tag="x", name="x_big")
                rhs_order = [("v", 0), ("v", 1), ("i", 0), ("i", 1)]
                for h in range(2):
                    hw_ = NBIG // 2
                    hs = slice(h * hw_, (h + 1) * hw_)
                    gs = slice(j * NBIG + h * hw_, j * NBIG + (h + 1) * hw_)
                    for nm, dram in (("v", vis), ("i", ir)):
                        for c in range(CCH):
                            nc.sync.dma_start(
                                ins[(nm, c)][:, hs],
                                dram[b, c * 128 : (c + 1) * 128, gs],
                            )
                    for c in range(CCH):
                        nc.vector.tensor_add(
                            outs[c][:, hs],
                            asf(ins[("v", c)][:, hs]),
                            asf(ins[("i", c)][:, hs]),
                        )
                    for js in (2 * h, 2 * h + 1):
                        sl = slice(js * NT, (js + 1) * NT)
                        ps1 = ps1pool.tile([HID1, NT], F32, tag="ps1", name="ps1")
                        for ko, key in enumerate(rhs_order):
                            nc.tensor.matmul(
                                ps1,
                                lhsT=w1t_sb[:, ko],
                                rhs=ins[key][:, sl],
                                start=(ko == 0),
                                stop=(ko == KO - 1),
                            )
                        nc.scalar.activation(
                            x_big[:, sl], ps1,
                            mybir.ActivationFunctionType.Relu,
                            bias=b1_sb[:, 0:1],
                        )
                    for c in range(CCH):
                        ps2 = ps2pool.tile([128, 2 * NT], F32, tag="ps2", name="ps2")
                        for q in range(2):
                            xsl = slice((2 * h + q) * NT, (2 * h + q + 1) * NT)
                            nc.tensor.matmul(
                                ps2[:, q * NT : (q + 1) * NT],
                                lhsT=w2t_sb[:, c * 128 : (c + 1) * 128],
                                rhs=x_big[:, xsl],
                                start=True,
                                stop=True,
                            )
                        nc.vector.tensor_add(outs[c][:, hs], outs[c][:, hs], ps2)
                        nc.scalar.dma_start(
                            out[b, c * 128 : (c + 1) * 128, gs], outs[c][:, hs]
                        )

            for b in range(B_PER_CORE):
                for j in range(HWPIX // NBIG):
                    if v6:
                        supertile_v6(b, j)
                        continue
                    jsl = slice(j * NBIG, (j + 1) * NBIG)
                    ins = {}
                    if halves:
                        # interleaved half-tile DMAs: the single SP queue is
                        # FIFO, so this ordering lands a complete half-set
                        # (all 4 inputs' columns h) every ~5.6us instead of
                        # one whole input every ~2.8us. PE can start each
                        # js-pair as its half-set arrives; idle slivers stay
                        # under the ~3.4us HAM window.
                        for nm, dram in (("v", vis), ("i", ir)):
                            for c in range(CCH):
                                ins[(nm, c)] = inpool.tile(
                                    [128, NBIG], d,
                                    tag=f"in_{nm}{c}", name=f"in_{nm}{c}",
                                )
                        # the very first supertile splits its input halves
                        # across both HWDGE queues (outputs don't need the
                        # ACT queue yet) so the first half-set lands in
                        # ~2.8us instead of ~5.6us
                        dual = tweak_ends and b == 0 and j == 0
                        for h in range(2):
                            hs = slice(h * (NBIG // 2), (h + 1) * (NBIG // 2))
                            gs = slice(
                                j * NBIG + h * (NBIG // 2),
                                j * NBIG + (h + 1) * (NBIG // 2),
                            )
                            for nm, dram in (("v", vis), ("i", ir)):
                                eng = nc.scalar if (dual and nm == "i") else nc.sync
                                for c in range(CCH):
                                    eng.dma_start(
                                        ins[(nm, c)][:, hs],
                                        dram[b, c * 128 : (c + 1) * 128, gs],
                                    )
                    else:
                        for nm, dram in (("v", vis), ("i", ir)):
                            for c in range(CCH):
                                t = inpool.tile(
                                    [128, NBIG], d,
                                    tag=f"in_{nm}{c}", name=f"in_{nm}{c}",
                                )
                                nc.sync.dma_start(
                                    t, dram[b, c * 128 : (c + 1) * 128, jsl]
                                )
                                ins[(nm, c)] = t
                    outs = [
                        opool.tile([128, NBIG], od, tag=f"out{c}", name=f"outt{c}")
                        for c in range(CCH)
                    ]
                    if big:
                        # bypass sum first: outs[c] = vis_c + ir_c in big DVE
                        # ops (fewer instructions, runs while PE does phase A)
                        for c in range(CCH):
                            if halves:
                                for h in range(2):
                                    hs = slice(
                                        h * (NBIG // 2), (h + 1) * (NBIG // 2)
                                    )
                                    nc.vector.tensor_add(
                                        outs[c][:, hs],
                                        asf(ins[("v", c)][:, hs]),
                                        asf(ins[("i", c)][:, hs]),
                                    )
                            else:
                                nc.vector.tensor_add(
                                    outs[c],
                                    asf(ins[("v", c)]),
                                    asf(ins[("i", c)]),
                                )
                    # Phase A: first layer for the whole supertile.
                    # x rows 0..31 = relu(W1@c + b1); row 32 = relu(0+1) = 1
                    x_big = wpool.tile([HID1, NBIG], d, tag="x", name="x_big")
                    rhs_order = [("v", 0), ("v", 1), ("i", 0), ("i", 1)]
                    if ko_outer:
                        # K-outer: the single HWDGE queue delivers the four
                        # input tiles sequentially (~2.8us apart), so consume
                        # each K chunk as it lands instead of waiting for all
                        # four. Keeps PE idle slivers under the ~3.4us HAM
                        # window -> matmuls stay at 2.4 GHz.
                        ps1s = [
                            ps1pool.tile([HID1, NT], F32, tag="ps1", name="ps1")
                            for _ in range(NBIG // NT)
                        ]
                        for ko, key in enumerate(rhs_order):
                            for js in range(NBIG // NT):
                                sl = slice(js * NT, (js + 1) * NT)
                                nc.tensor.matmul(
                                    ps1s[js],
                                    lhsT=w1t_sb[:, ko],
                                    rhs=ins[key][:, sl],
                                    start=(ko == 0),
                                    stop=(ko == KO - 1),
                                )
                        for js in range(NBIG // NT):
                            sl = slice(js * NT, (js + 1) * NT)
                            nc.scalar.activation(
                                x_big[:, sl], ps1s[js],
                                mybir.ActivationFunctionType.Relu,
                                bias=b1_sb[:, 0:1],
                            )
                    else:
                        for js in range(NBIG // NT):
                            sl = slice(js * NT, (js + 1) * NT)
                            ps1 = ps1pool.tile([HID1, NT], F32, tag="ps1", name="ps1")
                            for ko, key in enumerate(rhs_order):
                                nc.tensor.matmul(
                                    ps1,
                                    lhsT=w1t_sb[:, ko],
                                    rhs=ins[key][:, sl],
                                    start=(ko == 0),
                                    stop=(ko == KO - 1),
                                )
                            nc.scalar.activation(
                                x_big[:, sl], ps1, mybir.ActivationFunctionType.Relu,
                                bias=b1_sb[:, 0:1],
                            )
                    # Phase B: second layer + bypass adds.
                    if tweak_ends and (b, j) == last_bj:
                        # final supertile: NT-granular phase B so the kernel
                        # tail is one small chunk's matmul+add+DMA chain
                        for js in range(NBIG // NT):
                            sl = slice(js * NT, (js + 1) * NT)
                            for c in range(CCH):
                                ps2 = ps2pool.tile(
                                    [128, 2 * NT], F32, tag="ps2", name="ps2"
                                )
                                nc.tensor.matmul(
                                    ps2[:, :NT],
                                    lhsT=w2t_sb[:, c * 128 : (c + 1) * 128],
                                    rhs=x_big[:, sl],
                                    start=True,
                                    stop=True,
                                )
                                nc.vector.tensor_add(
                                    outs[c][:, sl], outs[c][:, sl], ps2[:, :NT]
                                )
                                nc.scalar.dma_start(
                                    out[
                                        b,
                                        c * 128 : (c + 1) * 128,
                                        j * NBIG + js * NT : j * NBIG
                                        + (js + 1) * NT,
                                    ],
                                    outs[c][:, sl],
                                )
                        continue
                    if big:
                        # mm2 in [128, 2*NT] psum chunks; one in-place DVE
                        # add per chunk (outs += residual+b2), then DMA the
                        # finished half straight out.
                        for h in range(NBIG // (2 * NT)):
                            hsl = slice(h * 2 * NT, (h + 1) * 2 * NT)
                            for c in range(CCH):
                                ps2 = ps2pool.tile(
                                    [128, 2 * NT], F32, tag="ps2", name="ps2"
                                )
                                for q in range(2):
                                    xsl = slice(
                                        (2 * h + q) * NT, (2 * h + q + 1) * NT
                                    )
                                    nc.tensor.matmul(
                                        ps2[:, q * NT : (q + 1) * NT],
                                        lhsT=w2t_sb[:, c * 128 : (c + 1) * 128],
                                        rhs=x_big[:, xsl],
                                        start=True,
                                        stop=True,
                                    )
                                nc.vector.tensor_add(
                                    outs[c][:, hsl], outs[c][:, hsl], ps2
                                )
                                # out-DMAs ride the ACT HWDGE queue so they
                                # never delay input prefetch on the SP queue
                                nc.scalar.dma_start(
                                    out[
                                        b,
                                        c * 128 : (c + 1) * 128,
                                        j * NBIG + h * 2 * NT : j * NBIG
                                        + (h + 1) * 2 * NT,
                                    ],
                                    outs[c][:, hsl],
                                )
                        continue
                    for js in range(NBIG // NT):
                        sl = slice(js * NT, (js + 1) * NT)
                        for c in range(CCH):
                            ps2 = ps2pool.tile(
                                [128, NT], F32, tag=f"ps2_{c}", name=f"ps2_{c}"
                            )
                            if hybrid:
                                # psum2 = I.T @ ir_c  (ir bypass on the PE)
                                nc.tensor.matmul(
                                    ps2,
                                    lhsT=iden_sb[:, :128],
                                    rhs=ins[("i", c)][:, sl],
                                    start=True,
                                    stop=False,
                                )
                            # psum2 += [W2; b2].T @ [x; 1] = residual + b2
                            nc.tensor.matmul(
                                ps2,
                                lhsT=w2t_sb[:, c * 128 : (c + 1) * 128],
                                rhs=x_big[:, sl],
                                start=not hybrid,
                                stop=True,
                            )
                            if hybrid:
                                nc.vector.tensor_add(
                                    outs[c][:, sl],
                                    ps2,
                                    asf(ins[("v", c)][:, sl]),
                                )
                            else:
                                s_t = wpool.tile([128, NT], F32, tag="s", name="s_t")
                                nc.vector.tensor_add(
                                    s_t,
                                    asf(ins[("v", c)][:, sl]),
                                    asf(ins[("i", c)][:, sl]),
                                )
                                nc.vector.tensor_add(outs[c][:, sl], ps2, s_t)
                    for c in range(CCH):
                        nc.sync.dma_start(
                            out[b, c * 128 : (c + 1) * 128, jsl], outs[c]
                        )
    nc.compile()
    return nc


def _build_v7(mm_dt: str) -> bass.Bass:
    """bf16 streaming rewrite.

    Structure (per core: 2 images, split into pixel ranges of NR):
      - One DMA push per (tensor, range): dram [256, NR] -> SBUF [128, 2, NR]
        via a (g p) x -> p g x access pattern. 16 input pushes + 8 output
        pushes total (fixed ~0.6us issue cost per push makes fewer/bigger
        pushes strictly better).
      - Every tile is uniquely allocated (SBUF is big enough at bf16 to hold
        ALL inputs + outputs) -> zero WAR hazards, so every input push is
        issued up front and the input stream runs at full DMA rate.
      - Per range: DVE bypass add (vis+ir, optionally split with gpsimd),
        ko-outer mm1 (4 LDWEIGHTS instead of 8), ACT relu+b1, mm2 with b2 via
        ones-row, DVE psum adds, one output push on the ACT queue.
      - Optional PE keep-warm dummy matmuls per range: the PE p-state drops
        to 1.2 GHz after any idle gap and needs 3us of continuous work to
        re-reach 2.4 GHz; dummies bridge the gaps left by the DMA cadence.
    """
    d = BF16
    NR = int(os.environ.get("KNR", "1024"))
    NCH = 512
    n_warm = int(os.environ.get("KWARM", "12"))
    n_dummy = int(os.environ.get("KDUMMY", "0"))
    gp_bypass = int(os.environ.get("KGP", "0"))
    ps1_bufs = int(os.environ.get("KPS1", "2"))
    ps2_bufs = int(os.environ.get("KPS2", "2"))
    split_ends = int(os.environ.get("KSPLIT", "0"))
    NRMAX = NR

    nc = bacc.Bacc(
        "TRN2", target_bir_lowering=False, debug=False, num_devices=N_CORES
    )
    vis = nc.dram_tensor("vis", [B_PER_CORE, C, HWPIX], d, kind="ExternalInput")
    ir = nc.dram_tensor("ir", [B_PER_CORE, C, HWPIX], d, kind="ExternalInput")
    w1t = nc.dram_tensor("w1t", [KO, 128, HID1], d, kind="ExternalInput")
    b1 = nc.dram_tensor("b1", [HID1, 1], F32, kind="ExternalInput")
    w2t = nc.dram_tensor("w2t", [HID1, C], d, kind="ExternalInput")
    iden = nc.dram_tensor("iden", [128, 4 * 128], d, kind="ExternalInput")
    out = nc.dram_tensor("out", [B_PER_CORE, C, HWPIX], d, kind="ExternalOutput")

    ranges = [
        (b, j * NR, NR) for b in range(B_PER_CORE) for j in range(HWPIX // NR)
    ]
    if split_ends:
        # halve the first and last ranges: faster pipeline fill and a
        # shorter serial drain chain at the kernel tail
        b0, j0, nr0 = ranges[0]
        ranges[0:1] = [(b0, j0, nr0 // 2), (b0, j0 + nr0 // 2, nr0 // 2)]
        bl, jl, nrl = ranges[-1]
        ranges[-1:] = [(bl, jl, nrl // 2), (bl, jl + nrl // 2, nrl // 2)]

    from contextlib import ExitStack

    with TileContext(nc) as tc, ExitStack() as es:
        if True:
            cpool = es.enter_context(tc.tile_pool(name="consts", bufs=1))
            dpool = es.enter_context(tc.tile_pool(name="data", bufs=1))
            ps1pool = es.enter_context(
                tc.tile_pool(name="ps1", bufs=ps1_bufs, space="PSUM")
            )
            ps2pool = es.enter_context(
                tc.tile_pool(name="ps2", bufs=ps2_bufs, space="PSUM")
            )

            # consts ride the ACT queue so the sync queue starts with real data
            iden_sb = cpool.tile([128, 4 * 128], d, tag="iden")
            nc.scalar.dma_start(iden_sb, iden[:, :])
            w1t_sb = cpool.tile([128, KO, HID1], d, tag="w1t")
            nc.scalar.dma_start(w1t_sb, w1t[:, :, :].rearrange("ko p m -> p ko m"))
            w2t_sb = cpool.tile([HID1, C], d, tag="w2t")
            nc.scalar.dma_start(w2t_sb, w2t[:, :])
            b1_sb = cpool.tile([HID1, 1], F32, tag="b1")
            nc.scalar.dma_start(b1_sb, b1[:, :])

            # all input pushes up front; unique buffers -> no WAR waits
            in_v, in_i = {}, {}
            for r, (b, j0, nr) in enumerate(ranges):
                jsl = slice(j0, j0 + nr)
                tv = dpool.tile([128, 2, nr], d, tag=f"v{r}", name=f"v{r}")
                ti = dpool.tile([128, 2, nr], d, tag=f"i{r}", name=f"i{r}")
                nc.sync.dma_start(
                    tv, vis[b, :, jsl].rearrange("(g p) x -> p g x", g=2)
                )
                nc.sync.dma_start(
                    ti, ir[b, :, jsl].rearrange("(g p) x -> p g x", g=2)
                )
                in_v[r], in_i[r] = tv, ti

            def pe_filler(n):
                # keep-warm matmuls through the ps1 rotation: their output is
                # never read, so the only dep is a free same-engine WAW
                for _ in range(n):
                    ps = ps1pool.tile([HID1, NRMAX], F32, tag="ps1", name="ps1w")
                    nc.tensor.matmul(
                        ps[:, :NCH],
                        lhsT=iden_sb[:, :HID1],
                        rhs=iden_sb[:, :NCH],
                        start=True,
                        stop=True,
                    )

            pe_filler(n_warm)

            for r, (b, j0, nr) in enumerate(ranges):
                jsl = slice(j0, j0 + nr)
                tv, ti = in_v[r], in_i[r]
                outt = dpool.tile([128, 2, nr], d, tag=f"o{r}", name=f"o{r}")
                if gp_bypass:
                    nc.vector.tensor_add(outt[:, 0], tv[:, 0], ti[:, 0])
                    nc.gpsimd.tensor_add(outt[:, 1], tv[:, 1], ti[:, 1])
                else:
                    nc.vector.tensor_add(outt, tv, ti)
                x_t = dpool.tile([HID1, nr], d, tag=f"x{r}", name=f"x{r}")
                nch = (nr + NCH - 1) // NCH
                ps1 = ps1pool.tile([HID1, NRMAX], F32, tag="ps1", name="ps1")
                for ko in range(KO):
                    src = tv if ko < 2 else ti
                    g = ko % 2
                    for ch in range(nch):
                        sl = slice(ch * NCH, min((ch + 1) * NCH, nr))
                        nc.tensor.matmul(
                            ps1[:, sl],
                            lhsT=w1t_sb[:, ko],
                            rhs=src[:, g, sl],
                            start=(ko == 0),
                            stop=(ko == KO - 1),
                        )
                nc.scalar.activation(
                    x_t,
                    ps1[:, :nr],
                    mybir.ActivationFunctionType.Relu,
                    bias=b1_sb[:, 0:1],
                )
                for c in range(CCH):
                    ps2 = ps2pool.tile([128, NRMAX], F32, tag="ps2", name="ps2")
                    for ch in range(nch):
                        sl = slice(ch * NCH, min((ch + 1) * NCH, nr))
                        nc.tensor.matmul(
                            ps2[:, sl],
                            lhsT=w2t_sb[:, c * 128 : (c + 1) * 128],
                            rhs=x_t[:, sl],
                            start=True,
                            stop=True,
                        )
                    nc.vector.tensor_add(outt[:, c], outt[:, c], ps2[:, :nr])
                pe_filler(n_dummy)
                nc.scalar.dma_start(
                    out[b, :, jsl].rearrange("(g p) x -> p g x", g=2), outt
                )
    nc.compile()
    return nc


def _get_nc(mm_dt: str, variant: str) -> bass.Bass:
    key = ("nc", mm_dt, variant)
    if key not in _cache:
        if variant == "v7":
            _cache[key] = _build_v7(mm_dt)
        else:
            _cache[key] = _build(mm_dt, variant)
    return _cache[key]


def kernel(
    visible_features: np.ndarray,
    infrared_features: np.ndarray,
    W1: np.ndarray,
    b1: np.ndarray,
    W2: np.ndarray,
    b2: np.ndarray,
    _mm_dt: str = "bf16",
    _variant: str = "v5",
    _trace: bool = False,
) -> np.ndarray:
    nc = _get_nc(_mm_dt, _variant)

    if _mm_dt == "bf16":
        import ml_dtypes

        io_np = ml_dtypes.bfloat16
    else:
        io_np = np.float32

    vis = np.ascontiguousarray(
        np.asarray(visible_features).astype(io_np).reshape(B, C, HWPIX)
    )
    ir = np.ascontiguousarray(
        np.asarray(infrared_features).astype(io_np).reshape(B, C, HWPIX)
    )

    w1t = np.zeros((2 * C, HID1), dtype=np.float32)
    w1t[:, :HID] = W1.astype(np.float32).T
    w1t = np.ascontiguousarray(w1t.reshape(KO, 128, HID1)).astype(io_np)
    b1r = np.ones((HID1, 1), dtype=np.float32)
    b1r[:HID, 0] = b1.astype(np.float32)
    w2t = np.zeros((HID1, C), dtype=np.float32)
    w2t[:HID] = W2.astype(np.float32).T
    w2t[HID] = b2.astype(np.float32)
    w2t = w2t.astype(io_np)
    iden = np.zeros((128, 4 * 128), dtype=np.float32)
    iden[:, :128] = np.eye(128, dtype=np.float32)
    iden = iden.astype(io_np)

    in_maps = []
    for core in range(N_CORES):
        bsl = slice(core * B_PER_CORE, (core + 1) * B_PER_CORE)
        in_maps.append(
            {
                "vis": vis[bsl],
                "ir": ir[bsl],
                "w1t": w1t,
                "b1": b1r,
                "w2t": w2t,
                "iden": iden,
            }
        )

    res = bass_utils.run_bass_kernel_spmd(
        nc, in_maps, core_ids=list(range(N_CORES)), trace=_trace
    )
    if _trace:
        kernel.last_results = res
    outs = [np.asarray(r["out"]).astype(np.float32) for r in res.results]
    return np.concatenate(outs, axis=0).reshape(B, C, H, W)

